# revision 1
# baseline (speedup 1.0000x reference)
"""Trainium2 Bass kernel for nn_DiscretisedBNF (discretised BNF loss).

Math reduction used on device: the reference's (B, D, K=128) clamped-CDF
bin sum collapses (Abel summation) to

    pO[b,d] = -127/256 + sum_{k=1..127} u_k * erf(z_k),
    z_k = (e_k - mu_x) * inv,   e_k = 2k/128 - 1,
    u_k = -1/128 (k<127),  u_127 = 125/256,
    inv = 1 / (sigma_x * sqrt(2))

erf is approximated on device by tanh(1.20331*z) (minimax fit, max abs
err 0.019; end-to-end loss rel err ~1.6e-3 incl. all quantization) so
that the whole kernel uses a single ACT table set (exp_and_others has
exp, tanh and leaky_relu; erf would force a ~2.7us table switch between
the exp and the binning phase).

Sharding (8 cores, full inputs in, full output out):
  - mm1 (mu_cat @ W1) replicated per core, fp8 DoubleRow (2 k-subtiles
    per matmul), with the t-row and b1 folded in as a K=2 bf16 matmul,
  - W2 column-sharded: core i owns output columns {i*128..} (mu_eps)
    and {1024+i*128..} (ln_sigma); mm2 fp8 DoubleRow + b2 ones-row,
  - binning data-parallel over the same d-slice: 32768 elements/core,
  - per-core output: 128 partial sums of sigma1^{-2t}*(x-pO)^2; host
    reduces and scales.

Inputs are host-packed into a few large SBUF-layout blobs so the input
pipe is ~15 large DMAs split across both HWDGE queues instead of ~40
small serialized ones. Element order for binning is dh-major
(g = dh*16384 + p64*256 + b, d_local = dh*64 + p64) so each half of the
prep (driven by one mu_eps/ln_sig half of mm2) feeds a contiguous run
of bin groups, letting ACT start tanh right behind mm2.
"""

import sys

sys.path.insert(0, "/opt/trn_rl_repo")

import numpy as np
import ml_dtypes

import concourse.bass as bass
import concourse.tile as tile
from concourse import bacc, mybir
from concourse.alu_op_type import AluOpType
from concourse.bass_utils import run_bass_kernel_spmd

B, D, H, K = 256, 1024, 2048, 128
NCORES = 8
DSL = D // NCORES  # 128 d-columns per core
SIGMA1 = 0.02
TMIN = 1e-10
LEAK = 0.01
C127 = 127.0 / 256.0
ATAN = 1.2033141525242548  # tanh(ATAN*z) ~= erf(z)

F32 = mybir.dt.float32
BF16 = mybir.dt.bfloat16
FP8 = mybir.dt.float8e4
BFNP = ml_dtypes.bfloat16
F8NP = ml_dtypes.float8_e4m3

HELEMS = DSL // 2 * B          # 16384 elements per dh half
RHEAD = 48 * B                 # 12288 = 8 groups of 1536 (partitions 0:48)
GROUPS = [1536] * 10 + [1024]  # per-half group sizes (sum = 16384)

# bb blob column offsets (bf16, 4 partitions)
BB_TV = 0         # [0:2, 0:256]   row0 = t, row1 = ones
BB_W1T = 256      # [0:2, 256:2304] row0 = W1[D,:], row1 = b1
BB_EDG = 2304     # [0:4, 2304:2432] edge matrix
BB_B2 = 2432      # [0:1, 2432:2688] b2[cols]
BB_ONE = 2688     # [0:1, 2688:2944] ones
BB_W = 2944

# f64 blob column offsets (f32, 64 partitions; 512-wide = tiled x2 over dh)
FO_MF, FO_BV, FO_RM, FO_CE, FO_XS, FO_NS = 0, 512, 1024, 1536, 2048, 2560
F64_W = 3072


def _build(debug=False):
    nc = bacc.Bacc("TRN2", target_bir_lowering=False, debug=False,
                   num_devices=NCORES)

    d_muT = nc.dram_tensor("muT8", (128, 8 * B), FP8, kind="ExternalInput")
    d_w1 = nc.dram_tensor("w1m", (128, 16 * 8 * 128), FP8,
                          kind="ExternalInput")
    d_w2 = nc.dram_tensor("w2m", (128, 16 * 2 * DSL), FP8,
                          kind="ExternalInput")
    d_bb = nc.dram_tensor("bb", (4, BB_W), BF16, kind="ExternalInput")
    d_f64 = nc.dram_tensor("f64", (64, F64_W), F32, kind="ExternalInput")
    d_f128 = nc.dram_tensor("f128", (128, 2 * B), F32, kind="ExternalInput")
    d_uv = nc.dram_tensor("uv", (128, 1), BF16, kind="ExternalInput")
    d_edg3 = nc.dram_tensor("edg3", (68, 128), BF16, kind="ExternalInput")
    d_part = nc.dram_tensor("part", (128, 1), F32, kind="ExternalOutput")

    MULT, ADD, SUB, BYP = (AluOpType.mult, AluOpType.add,
                           AluOpType.subtract, AluOpType.bypass)
    AF = mybir.ActivationFunctionType
    DR = mybir.MatmulPerfMode.DoubleRow

    with tile.TileContext(nc) as tc:
        with (
            tc.tile_pool(name="weights", bufs=1) as wpool,
            tc.tile_pool(name="work", bufs=1) as work,
            tc.tile_pool(name="stage", bufs=1) as stage,
        ):
            muT = wpool.tile([128, 8, B], FP8)
            w1s = [wpool.tile([128, 2, 8, 128], FP8, name=f"w1s{i}")
                   for i in range(8)]
            w2 = wpool.tile([128, 16, 2 * DSL], FP8)
            bb = wpool.tile([4, BB_W], BF16)
            f64 = wpool.tile([64, F64_W], F32)
            f128 = wpool.tile([128, 2 * B], F32)
            uv = wpool.tile([128, 1], BF16)
            edg3 = wpool.tile([68, 128], BF16)
            hT = work.tile([128, 16, B], FP8)

            with (
                tc.tile_pool(name="psA", bufs=3,
                             space=bass.MemorySpace.PSUM) as psA,
                tc.tile_pool(name="psO", bufs=1,
                             space=bass.MemorySpace.PSUM) as psO,
            ):
                # ---- input DMAs: sync (HWDGE) carries the mm1-critical
                # tensors; gpsimd (SWDGE) carries the rest. scalar/ACT
                # stays DMA-free so activations never queue behind DMAs.
                nc.sync.dma_start(muT[:], d_muT.ap()[:])
                nc.sync.dma_start(w1s[0][:], d_w1.ap()[:, 0:2048])
                nc.sync.dma_start(bb[:], d_bb.ap()[:])
                for s4 in range(1, 8):  # 2 m-tiles (256KB) per slab
                    nc.sync.dma_start(
                        w1s[s4][:], d_w1.ap()[:, s4 * 2048:(s4 + 1) * 2048])
                nc.sync.dma_start(f64[:], d_f64.ap()[:])
                nc.sync.dma_start(uv[:], d_uv.ap()[:])
                nc.sync.dma_start(edg3[:], d_edg3.ap()[:])
                nc.sync.dma_start(w2[:], d_w2.ap()[:])
                nc.sync.dma_start(f128[:], d_f128.ap()[:])

                # s = x*mf + (1-gamma)*mf*noise  (the mu/gamma term, masked)
                a1 = work.tile([64, 2, B], F32)
                nc.vector.tensor_tensor(
                    a1[:], f64[:, FO_XS:FO_XS + 512], f64[:, FO_MF:FO_MF + 512],
                    MULT)
                a2 = work.tile([64, 2, B], F32)
                nc.vector.tensor_tensor(
                    a2[:], f64[:, FO_NS:FO_NS + 512], f64[:, FO_BV:FO_BV + 512],
                    MULT)
                s = work.tile([64, 2, B], F32)
                nc.vector.tensor_tensor(s[:], a1[:], a2[:], ADD)
                # dummy exp: pull the exp_and_others ACT table load into
                # the mm1 window (tanh/exp later need no load)
                dum = work.tile([1, 1], F32)
                nc.scalar.activation(dum[:], bb[0:1, 0:1], AF.Exp,
                                     bias=0.0, scale=1.0)

                # ---- mm1: hT[m] = LeakyReLU(W1^T mu_cat^T) fp8 DoubleRow;
                # t-row and b1 folded in as a K=2 bf16 matmul
                for m in range(16):
                    ph = psA.tile([128, B], F32, tag="ph")
                    for j in range(4):
                        nc.tensor.matmul(
                            ph[:], w1s[m // 2][:, m % 2, 2 * j:2 * j + 2, :],
                            muT[:, 2 * j:2 * j + 2, :],
                            start=(j == 0), stop=False, perf_mode=DR)
                    ms = slice(BB_W1T + m * 128, BB_W1T + (m + 1) * 128)
                    nc.tensor.matmul(ph[:], bb[0:2, ms], bb[0:2, BB_TV:BB_TV + B],
                                     start=False, stop=True)
                    u = work.tile([128, B], F32, tag="lrelu_u", bufs=2)
                    nc.vector.tensor_copy(u[:], ph[:])
                    nc.vector.scalar_tensor_tensor(
                        hT[:, m, :], u[:], LEAK, u[:],
                        op0=MULT, op1=AluOpType.max)

                # ---- mm2: po[mo] = W2[:,cols]^T hT + b2, fp8 DoubleRow.
                # Emitted in two halves: tiles (2,0) -> prep half a ->
                # tiles (3,1) -> prep half b. The deferred tiles give the
                # PE real work during half-a's flatten DMAs, so it never
                # idles long enough for HAM to re-throttle the clock.
                po = {}

                def mm2_tile(mo):
                    pt = psO.tile([64, B], F32, tag=f"po{mo}")
                    po[mo] = pt
                    mos = slice(mo * 64, (mo + 1) * 64)
                    for j in range(8):
                        nc.tensor.matmul(pt[:], w2[:, 2 * j:2 * j + 2, mos],
                                         hT[:, 2 * j:2 * j + 2, :],
                                         start=(j == 0), stop=False,
                                         perf_mode=DR)
                    b2s = slice(BB_B2 + mo * 64, BB_B2 + (mo + 1) * 64)
                    nc.tensor.matmul(pt[:], bb[0:1, b2s],
                                     bb[0:1, BB_ONE:BB_ONE + B],
                                     start=False, stop=True)

                # ---- binning prep, per dh half -------------------------
                QT = [[stage.tile([64, B], BF16, name=f"QT{h}r{r}")
                       for r in range(4)] for h in range(2)]
                R1 = stage.tile([4, RHEAD], BF16, name="R1a")
                R2 = stage.tile([4, HELEMS - RHEAD], BF16, name="R2a")
                RAb = stage.tile([4, HELEMS], BF16, name="RAb")
                R3b = stage.tile([68, 11 * 512], BF16, name="R3b")
                R3a = stage.tile([68, 3 * 512], BF16, name="R3a")
                def flatten_row(hh, r):
                    if hh == 0:
                        # half a: head (36 partitions = 6 groups) first
                        # for an early z start, then the main piece
                        nc.sync.dma_start(R1[r:r + 1, :], QT[0][r][0:48, :])
                    else:
                        nc.sync.dma_start(RAb[r:r + 1, :], QT[1][r][0:64, :])

                def flatten_main(hh, r):
                    if hh == 0:
                        nc.sync.dma_start(R2[r:r + 1, :], QT[0][r][48:64, :])
                    # hh == 1 rows are written whole by flatten_row

                def flatten_scatter(hh):
                    # late-consumed pieces are scattered into per-row-group
                    # copies so their z matmuls run 3x packed: half a's
                    # main piece (walked last) and all of half b
                    if hh == 0:
                        rv = R2[:].rearrange("p (blk i) -> p blk i", i=512)
                        for h in range(3):
                            nblk = 2 if h == 2 else 3
                            nc.sync.dma_start(
                                R3a[32 * h:32 * h + 4, 0:nblk * 512],
                                rv[:, h:8:3, :])
                        return
                    rav = RAb[:].rearrange("p (blk i) -> p blk i", i=512)
                    for h in range(3):
                        nblk = 10 if h == 2 else 11
                        nc.sync.dma_start(
                            R3b[32 * h:32 * h + 4, 0:nblk * 512],
                            rav[:, h:32:3, :])

                inv = [None, None]

                def prep_half(hh):
                    lnm = work.tile([64, B], F32, tag=f"lnm{hh}")
                    nc.vector.tensor_tensor(lnm[:], po[2 + hh][:],
                                            f64[:, FO_MF + hh * B:FO_MF + hh * B + B],
                                            MULT)
                    ei = work.tile([64, B], F32, tag=f"ei{hh}")
                    nc.scalar.activation(ei[:], lnm[:], AF.Exp, bias=0.0,
                                         scale=-1.0)
                    iv = work.tile([64, B], F32, tag=f"inv{hh}")
                    inv[hh] = iv
                    nc.vector.tensor_tensor(
                        iv[:], ei[:], f64[:, FO_CE + hh * B:FO_CE + hh * B + B],
                        MULT)
                    nc.vector.tensor_copy(QT[hh][0][:], iv[:])      # ih
                    flatten_row(hh, 0)
                    nc.vector.tensor_tensor(QT[hh][1][:], iv[:],
                                            QT[hh][0][:], SUB)      # il
                    flatten_row(hh, 1)
                    a4 = work.tile([64, B], F32, tag=f"a4{hh}")
                    nc.vector.tensor_tensor(
                        a4[:], f64[:, FO_RM + hh * B:FO_RM + hh * B + B],
                        po[hh][:], MULT)
                    mu_x = work.tile([64, B], F32, tag=f"mux{hh}")
                    nc.vector.tensor_tensor(mu_x[:], s[:, hh, :], a4[:], SUB)
                    mx = work.tile([64, B], F32, tag=f"mx{hh}")
                    nc.vector.tensor_tensor(mx[:], mu_x[:], iv[:], MULT)
                    nc.vector.tensor_copy(QT[hh][2][:], mx[:])      # hi
                    flatten_row(hh, 2)
                    nc.vector.tensor_tensor(QT[hh][3][:], mx[:],
                                            QT[hh][2][:], SUB)      # lo
                    flatten_row(hh, 3)
                    if hh == 1:
                        # b's scatter chain is the long pole; its DMAs go
                        # first, then half-a's tail pieces
                        for r in range(4):
                            flatten_main(1, r)
                        flatten_scatter(1)
                        for r in range(4):
                            flatten_main(0, r)
                        flatten_scatter(0)

                mm2_tile(2)
                mm2_tile(0)
                prep_half(0)
                mm2_tile(3)
                mm2_tile(1)
                prep_half(1)

            # ---- binning main loop -------------------------------------
            with (
                tc.tile_pool(name="psZ", bufs=2,
                             space=bass.MemorySpace.PSUM) as psZ,
                tc.tile_pool(name="psQ", bufs=1,
                             space=bass.MemorySpace.PSUM) as psQ,
                tc.tile_pool(name="erf", bufs=3) as epool,
            ):
                q = psQ.tile([128, B], F32)
                # (hh, base, gel) walk, z matmuls emitted one group ahead
                # of tanh/q so the PE never idles waiting on ACT.
                # Order: half-a heads (unpacked, available first), then
                # half b (packed), then half a's scattered tail (packed).
                walk = []
                for hh in range(2):
                    base = 0
                    for gel in GROUPS:
                        walk.append((hh, base, gel))
                        base += gel
                # a-heads (unpacked) first, then packed b, then the packed
                # a-tail last so both scatter chains have maximum slack
                walk = walk[0:8] + walk[11:22] + walk[8:11]

                zts = {}

                def emit_z(gi):
                    hh, base, gel = walk[gi]
                    zt = psZ.tile([128, 1536], F32, tag="zt")
                    zts[gi] = zt
                    if hh == 0 and base < RHEAD:
                        for h in range(gel // 512):
                            off = base + h * 512
                            nc.tensor.matmul(
                                zt[:, h * 512:(h + 1) * 512], edg3[0:4, :],
                                R1[:, off:off + 512], start=True, stop=True)
                    elif hh == 1 and base < 2 * 1536:
                        # bridge b's scatter latency: first two b groups
                        # read RAb directly (block 3g+h at zt slice h,
                        # same element placement as the packed layout)
                        g = base // 1536
                        rv = RAb[:].rearrange("p (blk i) -> p blk i", i=512)
                        for h in range(gel // 512):
                            nc.tensor.matmul(
                                zt[:, h * 512:(h + 1) * 512], edg3[0:4, :],
                                rv[:, 3 * g + h, :], start=True, stop=True)
                    else:
                        src_t = R3b if hh == 1 else R3a
                        g = base // 1536 if hh == 1 else (base - RHEAD) // 1536
                        for h in range(gel // 512):
                            nc.tensor.matmul(
                                zt[:, h * 512:(h + 1) * 512],
                                edg3[32 * h:32 * h + 4, :],
                                src_t[32 * h:32 * h + 4,
                                      g * 512:(g + 1) * 512],
                                start=True, stop=True,
                                tile_position=(32 * h, 0))

                emit_z(0)
                for gi, (hh, base, gel) in enumerate(walk):
                    if gi + 1 < len(walk):
                        emit_z(gi + 1)
                    zt = zts.pop(gi)
                    et = epool.tile([128, 1536], FP8, tag="et")
                    nc.scalar.activation(et[:, 0:gel], zt[:, 0:gel],
                                         AF.Tanh, bias=0.0, scale=ATAN)
                    for j in range(gel // 128):
                        c = (hh * HELEMS + base) // 128 + j
                        nc.tensor.matmul(q[:, c:c + 1],
                                         et[:, j * 128:(j + 1) * 128],
                                         uv[:], start=True, stop=True)

                # tail: part = sum_cols (sqw*(xqc - q))^2
                e1 = work.tile([128, B], F32)
                nc.vector.tensor_tensor(e1[:], f128[:, 0:B], q[:], SUB)
                dw = work.tile([128, B], F32)
                nc.vector.tensor_tensor(dw[:], e1[:], f128[:, B:2 * B], MULT)
                dw2 = work.tile([128, B], F32)
                part = work.tile([128, 1], F32)
                nc.vector.scalar_tensor_tensor(dw2[:], dw[:], 1.0, dw[:],
                                               op0=BYP, op1=MULT,
                                               accum_out=part[:])
                nc.sync.dma_start(d_part.ap()[:], part[:])

    nc.compile()
    return nc


def host_prep(x, t, noise, W1, b1, W2, b2):
    """Build the per-core in_maps (host-side packing + tiny per-row math)."""
    f32 = np.float32
    tv = t[:, 0].astype(f32)
    gamma = (1.0 - np.power(f32(SIGMA1), f32(2.0) * tv)).astype(f32)
    low = tv < TMIN
    mf = np.where(low, f32(0.0), f32(1.0)).astype(f32)
    gsafe = np.where(gamma > 0, gamma, f32(1.0)).astype(f32)
    r = np.sqrt((1.0 - gsafe) / gsafe).astype(f32)
    rsafe = np.where(r > 0, r, f32(1.0)).astype(f32)
    bv = ((1.0 - gamma) * mf).astype(f32)
    rm = (r * mf).astype(f32)
    cexp = np.where(low, f32(1.0 / np.sqrt(2.0)),
                    (1.0 / (rsafe * np.sqrt(2.0))).astype(f32)).astype(f32)
    sqw = np.power(f32(SIGMA1), -tv).astype(f32)

    xT = np.ascontiguousarray(x.T, dtype=f32)
    nT = np.ascontiguousarray(noise.T, dtype=f32)
    g2 = (gamma * (1.0 - gamma)).astype(f32)
    muT8 = np.ascontiguousarray(
        (xT * gamma[None, :] + nT * g2[None, :]).astype(f32)
        .reshape(8, 128, B).transpose(1, 0, 2).reshape(128, 8 * B)
        .astype(F8NP))

    # w1m[p, (m*8+k)*128 + c] = W1[k*128+p, m*128+c]
    w1f = W1[:D].astype(f32).reshape(8, 128, 16, 128)
    w1m = np.ascontiguousarray(
        w1f.transpose(1, 2, 0, 3).reshape(128, 16 * 8 * 128).astype(F8NP))

    # bb blob
    bbv = np.zeros((4, BB_W), dtype=BFNP)
    bbv[0, BB_TV:BB_TV + B] = tv.astype(BFNP)
    bbv[1, BB_TV:BB_TV + B] = BFNP(1.0)
    bbv[0, BB_W1T:BB_W1T + H] = W1[D].astype(BFNP)
    bbv[1, BB_W1T:BB_W1T + H] = b1.astype(BFNP)
    e = (2.0 * np.arange(1, K) / K - 1.0).astype(f32)  # 127 edges, bf16-exact
    bbv[0, BB_ONE:BB_ONE + B] = BFNP(1.0)
    edg3 = np.zeros((68, 128), dtype=BFNP)
    for hb in (0, 32, 64):
        edg3[hb + 0, :127] = e.astype(BFNP)
        edg3[hb + 1, :127] = e.astype(BFNP)
        edg3[hb + 2, :127] = BFNP(-1.0)
        edg3[hb + 3, :127] = BFNP(-1.0)

    # f64 blob (per-batch broadcasts, tiled x2 over dh)
    f64v = np.zeros((64, F64_W), dtype=f32)
    for off, v in ((FO_MF, mf), (FO_BV, bv), (FO_RM, rm), (FO_CE, cexp)):
        f64v[:, off:off + 512] = np.tile(v, 2)[None, :]

    uvec = np.zeros((128, 1), dtype=BFNP)
    uvec[:126, 0] = BFNP(-1.0 / K)
    uvec[126, 0] = BFNP(125.0 / 256.0)  # exact in bf16

    # q layout index math: q column c = hh*128 + r, partition p.
    # half a (hh=0) is linear: elem = r*128+p -> p64 = elem//256.
    # half b went through the 3-way block scatter: chunk k = r-12g,
    # h = k//4, i = (k%4)*128+p -> p64 = (3g+h)*2 + i//256.
    p_idx = np.arange(128)[:, None]
    c_idx = np.arange(B)[None, :]
    hh = c_idx // 128
    r = c_idx % 128
    lin = r * 128 + p_idx
    p64_a = lin // B
    b_a = lin % B
    gg = np.minimum(r // 12, 10)
    k = r - gg * 12
    i = (k % 4) * 128 + p_idx
    p64_b = (3 * gg + k // 4) * 2 + i // 256
    b_b = i % 256
    # half a: heads (q cols 0..95) linear; tail (cols 96..127) scattered
    gg_a = np.minimum((r - 96) // 12, 2)
    k_a = r - 96 - gg_a * 12
    i_a = (k_a % 4) * 128 + p_idx
    p64_at = 48 + (3 * gg_a + k_a // 4) * 2 + i_a // 256
    b_at = i_a % 256
    p64 = np.where(hh == 0, np.where(r < 96, p64_a, p64_at), p64_b)
    b_i = np.where(hh == 0, np.where(r < 96, b_a, b_at), b_b)
    d_l = hh * 64 + p64
    sqwq = np.ascontiguousarray(sqw[b_i], dtype=f32)

    def to64(a128):
        # [128 d, 256 b] -> [64 p, 2, 256] with [p, dh, b] = a[dh*64+p, b]
        return np.ascontiguousarray(
            a128.reshape(2, 64, B).transpose(1, 0, 2).reshape(64, 2 * B))

    in_maps = []
    for i in range(NCORES):
        cols = np.concatenate([np.arange(i * DSL, (i + 1) * DSL),
                               1024 + np.arange(i * DSL, (i + 1) * DSL)])
        # w2m[p, k*256 + c] = W2[k*128+p, cols[c]]
        w2m = np.ascontiguousarray(
            W2[:, cols].astype(f32).reshape(16, 128, 2 * DSL)
            .transpose(1, 0, 2).reshape(128, 16 * 2 * DSL).astype(F8NP))
        bbi = bbv.copy()
        bbi[0, BB_B2:BB_B2 + 2 * DSL] = b2[cols].astype(BFNP)
        f64i = f64v.copy()
        f64i[:, FO_XS:FO_XS + 512] = to64(xT[i * DSL:(i + 1) * DSL])
        f64i[:, FO_NS:FO_NS + 512] = to64(nT[i * DSL:(i + 1) * DSL])
        f128 = np.empty((128, 2 * B), dtype=f32)
        f128[:, 0:B] = x[b_i, i * DSL + d_l].astype(f32) + f32(C127)
        f128[:, B:2 * B] = sqwq
        in_maps.append({
            "muT8": muT8, "w1m": w1m, "w2m": w2m, "bb": bbi,
            "f64": f64i, "f128": f128, "uv": uvec, "edg3": edg3,
        })
    return in_maps


_nc_cache = {}


def get_nc(debug=False):
    if debug not in _nc_cache:
        _nc_cache[debug] = _build(debug)
    return _nc_cache[debug]


def run_on_cores(inputs, trace=False, debug=False, tmpdir=None):
    nc = get_nc(debug)
    in_maps = host_prep(**inputs)
    res = run_bass_kernel_spmd(nc, in_maps, core_ids=list(range(NCORES)),
                               trace=trace, tmpdir=tmpdir)
    total = np.float32(0.0)
    for i in range(NCORES):
        total += res.results[i]["part"].astype(np.float32).sum()
    loss = np.float32(-np.log(np.float32(SIGMA1)) * total / np.float32(B * D))
    return loss, res


_reset_done = [False]


def _maybe_reset_device():
    # Clear a wedged NRT exec unit left by a previous process (observed
    # NRT_EXEC_UNIT_UNRECOVERABLE persisting across runs). Best-effort.
    if _reset_done[0]:
        return
    _reset_done[0] = True
    try:
        import os
        import ctypes
        so = "/opt/axon/libaxon_pjrt.so"
        if os.path.exists(so):
            import jax

            jax.devices()
            lib = ctypes.CDLL(so)
            lib.axon_reset.restype = ctypes.c_int64
            lib.axon_reset()
    except Exception:
        pass


def kernel(**inputs):
    _maybe_reset_device()
    inputs = {k: np.asarray(v) for k, v in inputs.items()}
    loss, _ = run_on_cores(inputs)
    return np.asarray(loss, dtype=np.float32)



# revision 6
# speedup vs baseline: 1.8164x; 1.8164x over previous
"""Trainium2 Bass kernel for nn_DiscretisedBNF (discretised BNF loss).

Math: the reference's (B, D, K=128) clamped-CDF bin sum Abel-collapses to

    pO[b,d] = -127/256 - (1/128)*Sigma + (127/256)*erf(z_127),
    Sigma   = sum_{k=1..127} erf(z_k),  z_k = (e_k - mu_x)*inv

and Sigma is a uniform-grid Riemann sum of erf, so by Poisson summation
it equals the midpoint integral up to O(exp(-pi^2/s^2)) (s = inv/64):

    Sigma ~= (1/s)[ z_b*erf(z_b) - z_a*erf(z_a) + (e^{-z_b^2}-e^{-z_a^2})/sqrt(pi) ]
    z_a = inv*(-0.9921875) - mu_x*inv,  z_b = inv*(0.9921875) - mu_x*inv

This replaces the whole 127-bin binning phase (4.2M tanh + z/q matmuls
per core) with ~5 ACT passes and ~15 vector ops per [128,256] tile.
erf is evaluated as tanh((2/sqrt(pi))*(z + c*z^3)) (max abs err 3.6e-4),
so exp+tanh+square+prelu all live in the one resident ACT table set
(exp_and_others) -- no table switches.  End-to-end numpy mirror of the
device math (incl. fp8/bf16 quantization): rel err ~8e-5.

Constant foldings: mm2's ln-tile bias rows add -ln(cexp) (hi/lo bf16
split) so that  inv = exp(-PLN)  and  1/(128 s) = exp(PLN - ln2)  come
straight out of ACT with scalar biases; 1/sqrt(pi) is folded into the
exp bias.

Sharding (8 cores, full inputs in, full output out): mm1 replicated
(fp8 DoubleRow), W2 column-sharded 128+128 cols per core, epilogue
data-parallel on the core's [128 d x 256 b] tile. Output is a single
f32 partial per core (cross-partition reduce via a ones-matmul) so the
final DMA is one 4-byte descriptor. Host sums 8 partials.

PE warm-up: ~8 junk N=512 matmuls on a memset tile right at kernel
start keep HAM from running mm1 at the cold 1.2 GHz clock.
"""

import sys

sys.path.insert(0, "/opt/trn_rl_repo")

import numpy as np
import ml_dtypes

import concourse.bass as bass
import concourse.tile as tile
from concourse import bacc, mybir
from concourse.alu_op_type import AluOpType
from concourse.bass_utils import run_bass_kernel_spmd

B, D, H = 256, 1024, 2048
NCORES = 8
DSL = D // NCORES  # 128 d-columns per core
SIGMA1 = 0.02

F32 = mybir.dt.float32
BF16 = mybir.dt.bfloat16
FP8 = mybir.dt.float8e4
BFNP = ml_dtypes.bfloat16
F8NP = ml_dtypes.float8_e4m3

ERFA = float(2.0 / np.sqrt(np.pi))      # tanh scale
ERFC = float(0.10091075 / ERFA)          # z^3 coefficient (fit, err 3.6e-4)
LNPI2 = float(0.5 * np.log(np.pi))       # folded into exp(-z^2) bias
LN2 = float(np.log(2.0))

# bb blob (bf16, 4 partitions) column offsets
BB_TV = 0        # [0:2, 0:256]      row0 = t, row1 = ones (mm1 bias rhs)
BB_W1T = 256     # [0:2, 256:2304]   row0 = W1[D,:], row1 = b1 (mm1 bias lhsT)
BB_LNW = 2304    # [0:3, 2304:2432]  LN bias lhsT: [b2ln; 1; 1]
BB_LNR = 2432    # [0:3, 2432:2688]  LN bias rhs: [1; -lnCE_hi; -lnCE_lo]
BB_MUW = 2688    # [0:1, 2688:2816]  MU bias lhsT: b2mu
BB_MUR = 2816    # [0:1, 2816:3072]  MU bias rhs: ones
BB_W = 3072

# ep blob (f32, 128 partitions) column offsets
EP_S, EP_RM, EP_XC, EP_SQW, EP_ONE = 0, 256, 512, 768, 1024
EP_BLN2, EP_BPI = 1025, 1026   # bias columns: -ln2, -0.5*ln(pi)
EP_W = 1027


def _build(debug=False):
    nc = bacc.Bacc("TRN2", target_bir_lowering=False, debug=False,
                   num_devices=NCORES)

    d_muT = nc.dram_tensor("muT8", (128, 8 * B), FP8, kind="ExternalInput")
    d_w1 = nc.dram_tensor("w1m", (128, 16 * 8 * 128), FP8,
                          kind="ExternalInput")
    d_w2 = nc.dram_tensor("w2m", (128, 2 * 8 * 2 * 128), FP8,
                          kind="ExternalInput")
    d_bb = nc.dram_tensor("bb", (4, BB_W), BF16, kind="ExternalInput")
    d_ep = nc.dram_tensor("ep", (128, EP_W), F32, kind="ExternalInput")
    d_out = nc.dram_tensor("outp", (1, 1), F32, kind="ExternalOutput")

    MULT, ADD, SUB, BYP = (AluOpType.mult, AluOpType.add,
                           AluOpType.subtract, AluOpType.bypass)
    AF = mybir.ActivationFunctionType
    DR = mybir.MatmulPerfMode.DoubleRow

    with tile.TileContext(nc) as tc:
        with (
            tc.tile_pool(name="weights", bufs=1) as wpool,
            tc.tile_pool(name="work", bufs=1) as work,
        ):
            muT = wpool.tile([128, 8, B], FP8)
            w1s = [wpool.tile([128, 2, 8, 128], FP8, name=f"w1s{i}")
                   for i in range(8)]
            w2 = wpool.tile([128, 2, 8, 2, 128], FP8)
            bb = wpool.tile([4, BB_W], BF16)
            eps = wpool.tile([128, EP_W], F32)
            jw = wpool.tile([128, 640], BF16)
            hT = work.tile([128, 16, B], FP8)

            with (
                tc.tile_pool(name="psJ", bufs=1,
                             space=bass.MemorySpace.PSUM) as psJ,
                tc.tile_pool(name="psA", bufs=3,
                             space=bass.MemorySpace.PSUM) as psA,
                tc.tile_pool(name="psO", bufs=1,
                             space=bass.MemorySpace.PSUM) as psO,
            ):
                # ---- input DMAs: sync (HWDGE) carries the mm1-critical
                # tensors in need-order; gpsimd (SWDGE, ~25ns issue)
                # carries the rest concurrently.
                nc.gpsimd.memset(jw[:], 0.0)
                nc.gpsimd.dma_start(w2[:], d_w2.ap()[:])
                nc.gpsimd.dma_start(eps[:], d_ep.ap()[:])
                nc.sync.dma_start(muT[:], d_muT.ap()[:])
                nc.sync.dma_start(w1s[0][:], d_w1.ap()[:, 0:2048])
                nc.sync.dma_start(bb[:], d_bb.ap()[:])
                for s4 in range(1, 8):
                    nc.sync.dma_start(
                        w1s[s4][:], d_w1.ap()[:, s4 * 2048:(s4 + 1) * 2048])

                # ---- PE warm-up: dense junk matmuls (no DMA deps) so HAM
                # lifts the 1.2GHz cold clock before mm1's data lands.
                jp = psJ.tile([128, 512], F32)
                for _ in range(8):
                    nc.tensor.matmul(jp[:], jw[:, 0:128], jw[:, 128:640],
                                     start=True, stop=True)

                # ---- mm1: hT[m] = PRelu(W1^T mu_cat^T) fp8 DoubleRow;
                # t-row and b1 folded in as a K=2 bf16 matmul.
                for m in range(16):
                    ph = psA.tile([128, B], F32, tag="ph")
                    for j in range(4):
                        nc.tensor.matmul(
                            ph[:], w1s[m // 2][:, m % 2, 2 * j:2 * j + 2, :],
                            muT[:, 2 * j:2 * j + 2, :],
                            start=(j == 0), stop=False, perf_mode=DR)
                    ms = slice(BB_W1T + m * 128, BB_W1T + (m + 1) * 128)
                    nc.tensor.matmul(ph[:], bb[0:2, ms],
                                     bb[0:2, BB_TV:BB_TV + B],
                                     start=False, stop=True)
                    nc.scalar.activation(hT[:, m, :], ph[:], AF.Prelu,
                                         bias=0.0, scale=1.0, alpha=0.01)

                # ---- mm2: PLN = W2ln^T hT + b2ln - lnCE (hi/lo bf16 rows),
                # then PMU = W2mu^T hT + b2mu. M=128, fp8 DoubleRow.
                pln = psO.tile([128, B], F32, name="pln")
                for j in range(8):
                    nc.tensor.matmul(pln[:], w2[:, 1, j, :, :],
                                     hT[:, 2 * j:2 * j + 2, :],
                                     start=(j == 0), stop=False, perf_mode=DR)
                nc.tensor.matmul(pln[:], bb[0:3, BB_LNW:BB_LNW + 128],
                                 bb[0:3, BB_LNR:BB_LNR + B],
                                 start=False, stop=True)
                pmu = psO.tile([128, B], F32, name="pmu")
                for j in range(8):
                    nc.tensor.matmul(pmu[:], w2[:, 0, j, :, :],
                                     hT[:, 2 * j:2 * j + 2, :],
                                     start=(j == 0), stop=False, perf_mode=DR)
                nc.tensor.matmul(pmu[:], bb[0:1, BB_MUW:BB_MUW + 128],
                                 bb[0:1, BB_MUR:BB_MUR + B],
                                 start=False, stop=True)

                # ---- epilogue -----------------------------------------
                inv = work.tile([128, B], F32, name="inv")
                nc.scalar.activation(inv[:], pln[:], AF.Exp,
                                     bias=0.0, scale=-1.0)
                isp = work.tile([128, B], F32, name="isp")
                nc.scalar.activation(isp[:], pln[:], AF.Exp,
                                     bias=eps[:, EP_BLN2:EP_BLN2 + 1],
                                     scale=1.0)

                a4 = work.tile([128, B], F32, name="a4")
                nc.vector.tensor_tensor(a4[:], eps[:, EP_RM:EP_RM + B],
                                        pmu[:], MULT)
                md = work.tile([128, B], F32, name="md")
                nc.vector.tensor_tensor(md[:], eps[:, EP_S:EP_S + B],
                                        a4[:], SUB)
                mx = work.tile([128, B], F32, name="mx")
                nc.vector.tensor_tensor(mx[:], md[:], inv[:], MULT)

                zab = work.tile([128, 3, B], F32, name="zab")
                nc.vector.scalar_tensor_tensor(
                    zab[:, 0, :], inv[:], -0.9921875, mx[:],
                    op0=MULT, op1=SUB)
                nc.vector.scalar_tensor_tensor(
                    zab[:, 1, :], inv[:], 0.9921875, mx[:],
                    op0=MULT, op1=SUB)
                nc.vector.scalar_tensor_tensor(
                    zab[:, 2, :], inv[:], 0.984375, mx[:],
                    op0=MULT, op1=SUB)

                sq = work.tile([128, 3, B], F32, name="sq")
                nc.scalar.activation(sq[:], zab[:], AF.Square,
                                     bias=0.0, scale=1.0)
                cu = work.tile([128, 3, B], F32, name="cu")
                nc.vector.tensor_tensor(cu[:], sq[:], zab[:], MULT)
                uu = work.tile([128, 3, B], F32, name="uu")
                nc.vector.scalar_tensor_tensor(uu[:], cu[:], ERFC, zab[:],
                                               op0=MULT, op1=ADD)
                erf = work.tile([128, 3, B], F32, name="erf")
                nc.scalar.activation(erf[:], uu[:], AF.Tanh,
                                     bias=0.0, scale=ERFA)
                exg = work.tile([128, 2, B], F32, name="exg")
                nc.scalar.activation(exg[:], sq[:, 0:2, :], AF.Exp,
                                     bias=eps[:, EP_BPI:EP_BPI + 1],
                                     scale=-1.0)

                a1 = work.tile([128, B], F32, name="a1")
                nc.vector.tensor_tensor(a1[:], zab[:, 1, :], erf[:, 1, :],
                                        MULT)
                b1t = work.tile([128, B], F32, name="b1t")
                nc.gpsimd.tensor_tensor(b1t[:], zab[:, 0, :], erf[:, 0, :],
                                        MULT)
                c1 = work.tile([128, B], F32, name="c1")
                nc.gpsimd.tensor_tensor(c1[:], exg[:, 1, :], exg[:, 0, :],
                                        SUB)
                d1 = work.tile([128, B], F32, name="d1")
                nc.vector.tensor_tensor(d1[:], a1[:], b1t[:], SUB)
                e1 = work.tile([128, B], F32, name="e1")
                nc.vector.tensor_tensor(e1[:], d1[:], c1[:], ADD)
                sg = work.tile([128, B], F32, name="sg")
                nc.vector.tensor_tensor(sg[:], e1[:], isp[:], MULT)
                gg = work.tile([128, B], F32, name="gg")
                nc.vector.tensor_tensor(gg[:], sg[:], eps[:, EP_XC:EP_XC + B],
                                        ADD)
                hh = work.tile([128, B], F32, name="hh")
                nc.vector.scalar_tensor_tensor(
                    hh[:], erf[:, 2, :], 127.0 / 256.0, gg[:],
                    op0=MULT, op1=SUB)
                hs = work.tile([128, B], F32, name="hs")
                nc.vector.tensor_tensor(hs[:], hh[:],
                                        eps[:, EP_SQW:EP_SQW + B], MULT)
                h2 = work.tile([128, B], F32, name="h2")
                part = work.tile([128, 1], F32, name="part")
                nc.vector.scalar_tensor_tensor(h2[:], hs[:], 1.0, hs[:],
                                               op0=BYP, op1=MULT,
                                               accum_out=part[:])

                # cross-partition reduce: 1x1 psum via ones-matmul, so the
                # output DMA is a single 4-byte descriptor.
                ps1 = psO.tile([1, 1], F32, name="ps1")
                nc.tensor.matmul(ps1[:], part[:, 0:1],
                                 eps[:, EP_ONE:EP_ONE + 1],
                                 start=True, stop=True)
                sres = work.tile([1, 1], F32, name="sres")
                nc.vector.tensor_copy(sres[:], ps1[:])
                nc.sync.dma_start(d_out.ap()[:], sres[:])

    nc.compile()
    return nc


def host_prep(x, t, noise, W1, b1, W2, b2):
    """Build the per-core in_maps (host-side packing + tiny per-row math)."""
    f32 = np.float32
    tv = t[:, 0].astype(f32)
    assert float(tv.min()) > 1e-8, "low-t mask path not implemented"
    gamma = (1.0 - np.power(f32(SIGMA1), f32(2.0) * tv)).astype(f32)
    assert float(gamma.min()) > 0.0
    r = np.sqrt((1.0 - gamma) / gamma).astype(f32)
    lnce = np.log(1.0 / (r * np.sqrt(f32(2.0)))).astype(f32)
    nlh = (-lnce).astype(BFNP)
    nll = ((-lnce) - nlh.astype(f32)).astype(BFNP)
    sqw = np.power(f32(SIGMA1), -tv).astype(f32)

    mu = (gamma[:, None] * x + (gamma * (1 - gamma))[:, None] * noise
          ).astype(f32)
    muT8 = np.ascontiguousarray(
        mu.T.reshape(8, 128, B).transpose(1, 0, 2).reshape(128, 8 * B)
        .astype(F8NP))

    # w1m[p, (m*8+k)*128 + c] = W1[k*128+p, m*128+c]
    w1f = W1[:D].astype(f32).reshape(8, 128, 16, 128)
    w1m = np.ascontiguousarray(
        w1f.transpose(1, 2, 0, 3).reshape(128, 16 * 8 * 128).astype(F8NP))

    bbv = np.zeros((4, BB_W), dtype=BFNP)
    bbv[0, BB_TV:BB_TV + B] = tv.astype(BFNP)
    bbv[1, BB_TV:BB_TV + B] = BFNP(1.0)
    bbv[0, BB_W1T:BB_W1T + H] = W1[D].astype(BFNP)
    bbv[1, BB_W1T:BB_W1T + H] = b1.astype(BFNP)
    bbv[1, BB_LNW:BB_LNW + 128] = BFNP(1.0)
    bbv[2, BB_LNW:BB_LNW + 128] = BFNP(1.0)
    bbv[0, BB_LNR:BB_LNR + B] = BFNP(1.0)
    bbv[1, BB_LNR:BB_LNR + B] = nlh
    bbv[2, BB_LNR:BB_LNR + B] = nll
    bbv[0, BB_MUR:BB_MUR + B] = BFNP(1.0)

    epv = np.zeros((128, EP_W), dtype=f32)
    epv[:, EP_RM:EP_RM + B] = r[None, :]
    epv[:, EP_SQW:EP_SQW + B] = sqw[None, :]
    epv[:, EP_ONE] = 1.0
    epv[:, EP_BLN2] = -LN2
    epv[:, EP_BPI] = -LNPI2

    S_full = (x + (1.0 - gamma)[:, None] * noise).astype(f32)
    XC_full = (x + f32(127.0 / 256.0)).astype(f32)

    in_maps = []
    for i in range(NCORES):
        dsl = slice(i * DSL, (i + 1) * DSL)
        # w2m[p, ((half*8+j)*2+rr)*128+c] = W2[(2j+rr)*128+p, half*D + col]
        w2c = np.stack([W2[:, dsl], W2[:, D + i * DSL:D + (i + 1) * DSL]],
                       axis=0)  # [2, 2048, 128]
        w2m = np.ascontiguousarray(
            w2c.reshape(2, 16, 128, 128).transpose(2, 0, 1, 3)
            .reshape(128, 2 * 16 * 128).astype(F8NP))
        bbi = bbv.copy()
        bbi[0, BB_LNW:BB_LNW + 128] = b2[D + i * DSL:D + (i + 1) * DSL
                                         ].astype(BFNP)
        bbi[0, BB_MUW:BB_MUW + 128] = b2[dsl].astype(BFNP)
        epi = epv.copy()
        epi[:, EP_S:EP_S + B] = S_full[:, dsl].T
        epi[:, EP_XC:EP_XC + B] = XC_full[:, dsl].T
        in_maps.append({
            "muT8": muT8, "w1m": w1m, "w2m": w2m, "bb": bbi, "ep": epi,
        })
    return in_maps


_nc_cache = {}


def get_nc(debug=False):
    if debug not in _nc_cache:
        _nc_cache[debug] = _build(debug)
    return _nc_cache[debug]


def run_on_cores(inputs, trace=False, debug=False, tmpdir=None):
    nc = get_nc(debug)
    in_maps = host_prep(**inputs)
    res = run_bass_kernel_spmd(nc, in_maps, core_ids=list(range(NCORES)),
                               trace=trace, tmpdir=tmpdir)
    total = np.float64(0.0)
    for i in range(NCORES):
        total += np.float64(res.results[i]["outp"].reshape(-1)[0])
    loss = np.float32(-np.log(np.float32(SIGMA1)) * total / float(B * D))
    return loss, res


_reset_done = [False]


def _maybe_reset_device():
    # Clear a wedged NRT exec unit left by a previous process. Best-effort.
    if _reset_done[0]:
        return
    _reset_done[0] = True
    try:
        import os
        import ctypes
        so = "/opt/axon/libaxon_pjrt.so"
        if os.path.exists(so):
            import jax

            jax.devices()
            lib = ctypes.CDLL(so)
            lib.axon_reset.restype = ctypes.c_int64
            lib.axon_reset()
    except Exception:
        pass


def kernel(**inputs):
    _maybe_reset_device()
    inputs = {k: np.asarray(v) for k, v in inputs.items()}
    loss, _ = run_on_cores(inputs)
    return np.asarray(loss, dtype=np.float32)


# revision 11
# speedup vs baseline: 1.8515x; 1.0193x over previous
"""Trainium2 Bass kernel for nn_DiscretisedBNF (discretised BNF loss).

Math: the reference's (B, D, K=128) clamped-CDF bin sum Abel-collapses to

    pO[b,d] = -127/256 - (1/128)*Sigma + (127/256)*erf(z_127),
    Sigma   = sum_{k=1..127} erf(z_k),  z_k = (e_k - mu_x)*inv

and Sigma is a uniform-grid Riemann sum of erf, so by Poisson summation
it equals the midpoint integral up to O(exp(-pi^2/s^2)) (s = inv/64):

    Sigma ~= (1/s)[ z_b*erf(z_b) - z_a*erf(z_a) + (e^{-z_b^2}-e^{-z_a^2})/sqrt(pi) ]
    z_a = inv*(-0.9921875) - mu_x*inv,  z_b = inv*(0.9921875) - mu_x*inv

This replaces the whole 127-bin binning phase (4.2M tanh + z/q matmuls
per core) with ~5 ACT passes and ~15 vector ops per [128,256] tile.
erf is evaluated as tanh((2/sqrt(pi))*(z + c*z^3)) (max abs err 3.6e-4),
so exp+tanh+square+prelu all live in the one resident ACT table set
(exp_and_others) -- no table switches.  End-to-end numpy mirror of the
device math (incl. fp8/bf16 quantization): rel err ~8e-5.

Constant foldings: mm2's ln-tile bias rows add -ln(cexp) (hi/lo bf16
split) so that  inv = exp(-PLN)  and  1/(128 s) = exp(PLN - ln2)  come
straight out of ACT with scalar biases; 1/sqrt(pi) is folded into the
exp bias.

Sharding (8 cores, full inputs in, full output out): mm1 replicated
(fp8 DoubleRow), W2 column-sharded 128+128 cols per core, epilogue
data-parallel on the core's [128 d x 256 b] tile. Output is a single
f32 partial per core (cross-partition reduce via a ones-matmul) so the
final DMA is one 4-byte descriptor. Host sums 8 partials.

PE warm-up: ~8 junk N=512 matmuls on a memset tile right at kernel
start keep HAM from running mm1 at the cold 1.2 GHz clock.
"""

import sys

sys.path.insert(0, "/opt/trn_rl_repo")

import numpy as np
import ml_dtypes

import concourse.bass as bass
import concourse.tile as tile
from concourse import bacc, mybir
from concourse.alu_op_type import AluOpType
from concourse.bass_utils import run_bass_kernel_spmd

B, D, H = 256, 1024, 2048
NCORES = 8
DSL = D // NCORES  # 128 d-columns per core
SIGMA1 = 0.02

F32 = mybir.dt.float32
BF16 = mybir.dt.bfloat16
FP8 = mybir.dt.float8e4
BFNP = ml_dtypes.bfloat16
F8NP = ml_dtypes.float8_e4m3

ERFA = float(2.0 / np.sqrt(np.pi))      # tanh scale
ERFC = float(0.10091075 / ERFA)          # z^3 coefficient (fit, err 3.6e-4)
LNPI2 = float(0.5 * np.log(np.pi))       # folded into exp(-z^2) bias
LN2 = float(np.log(2.0))

# bb blob (bf16, 4 partitions) column offsets
BB_TV = 0        # [0:2, 0:256]      row0 = t, row1 = ones (mm1 bias rhs)
BB_W1T = 256     # [0:2, 256:2304]   row0 = W1[D,:], row1 = b1 (mm1 bias lhsT)
BB_LNW = 2304    # [0:3, 2304:2432]  LN bias lhsT: [b2ln; 1; 1]
BB_LNR = 2432    # [0:3, 2432:2688]  LN bias rhs: [1; -lnCE_hi; -lnCE_lo]
BB_MUW = 2688    # [0:1, 2688:2816]  MU bias lhsT: b2mu
BB_MUR = 2816    # [0:1, 2816:3072]  MU bias rhs: ones
BB_W = 3072

# ep blob (f32, 128 partitions) column offsets
EP_S, EP_RM, EP_XC, EP_SQW, EP_ONE = 0, 256, 512, 768, 1024
EP_BLN2, EP_BPI = 1025, 1026   # bias columns: -ln2, -0.5*ln(pi)
EP_W = 1027


def _build(debug=False):
    nc = bacc.Bacc("TRN2", target_bir_lowering=False, debug=False,
                   num_devices=NCORES)

    d_muT = nc.dram_tensor("muT8", (128, 8 * B), FP8, kind="ExternalInput")
    d_w1 = nc.dram_tensor("w1m", (128, 16 * 8 * 128), FP8,
                          kind="ExternalInput")
    d_w2 = nc.dram_tensor("w2m", (128, 2 * 8 * 2 * 128), FP8,
                          kind="ExternalInput")
    d_bb = nc.dram_tensor("bb", (4, BB_W), BF16, kind="ExternalInput")
    d_ep = nc.dram_tensor("ep", (128, EP_W), F32, kind="ExternalInput")
    d_out = nc.dram_tensor("outp", (2, 1), F32, kind="ExternalOutput")

    MULT, ADD, SUB, BYP = (AluOpType.mult, AluOpType.add,
                           AluOpType.subtract, AluOpType.bypass)
    AF = mybir.ActivationFunctionType
    DR = mybir.MatmulPerfMode.DoubleRow

    with tile.TileContext(nc) as tc:
        with (
            tc.tile_pool(name="weights", bufs=1) as wpool,
            tc.tile_pool(name="work", bufs=1) as work,
        ):
            muT = wpool.tile([128, 8, B], FP8)
            w1s = [wpool.tile([128, 2, 8, 128], FP8, name=f"w1s{i}")
                   for i in range(8)]
            w2 = wpool.tile([128, 2, 8, 2, 128], FP8)
            bb = wpool.tile([4, BB_W], BF16)
            eps = wpool.tile([128, EP_W], F32)
            jw = wpool.tile([128, 640], BF16)
            hT = work.tile([128, 16, B], FP8)

            with (
                tc.tile_pool(name="psJ", bufs=1,
                             space=bass.MemorySpace.PSUM) as psJ,
                tc.tile_pool(name="psA", bufs=3,
                             space=bass.MemorySpace.PSUM) as psA,
                tc.tile_pool(name="psO", bufs=1,
                             space=bass.MemorySpace.PSUM) as psO,
            ):
                # ---- input DMAs: sync (HWDGE) carries the mm1-critical
                # tensors in need-order. The mm2/epilogue tensors go on
                # gpsimd (SWDGE) but are deferred behind mm1's first tile
                # (dummy dep below) so they don't steal DMA-engine
                # bandwidth from the mm1-critical transfers.
                nc.gpsimd.memset(jw[:], 0.0)
                nc.sync.dma_start(w1s[0][:], d_w1.ap()[:, 0:2048])
                nc.sync.dma_start(muT[:], d_muT.ap()[:])
                nc.sync.dma_start(bb[:], d_bb.ap()[:])
                for s4 in range(1, 8):
                    nc.sync.dma_start(
                        w1s[s4][:], d_w1.ap()[:, s4 * 2048:(s4 + 1) * 2048])

                # ---- PE warm-up: dense junk matmuls (no DMA deps) so HAM
                # lifts the 1.2GHz cold clock before mm1's data lands.
                jp = psJ.tile([128, 512], F32)
                for _ in range(6):
                    nc.tensor.matmul(jp[:], jw[:, 0:128], jw[:, 128:640],
                                     start=True, stop=True)

                # ---- mm1: hT[m] = PRelu(W1^T mu_cat^T) fp8 DoubleRow;
                # t-row and b1 folded in as a K=2 bf16 matmul.
                for m in range(16):
                    ph = psA.tile([128, B], F32, tag="ph")
                    for j in range(4):
                        nc.tensor.matmul(
                            ph[:], w1s[m // 2][:, m % 2, 2 * j:2 * j + 2, :],
                            muT[:, 2 * j:2 * j + 2, :],
                            start=(j == 0), stop=False, perf_mode=DR)
                    ms = slice(BB_W1T + m * 128, BB_W1T + (m + 1) * 128)
                    nc.tensor.matmul(ph[:], bb[0:2, ms],
                                     bb[0:2, BB_TV:BB_TV + B],
                                     start=False, stop=True)
                    nc.scalar.activation(hT[:, m, :], ph[:], AF.Prelu,
                                         bias=0.0, scale=1.0, alpha=0.01)
                    if m == 0:
                        # dummy read of hT[0] delays the SWDGE input DMAs
                        # until the mm1-critical sync transfers are done
                        jd = work.tile([1, 1], FP8, name="jd")
                        nc.gpsimd.tensor_copy(jd[:], hT[0:1, 0, 0:1])
                        nc.gpsimd.dma_start(w2[:], d_w2.ap()[:])
                        nc.gpsimd.dma_start(eps[:], d_ep.ap()[:])

                # ---- mm2: PLN = W2ln^T hT + b2ln - lnCE (hi/lo bf16 rows),
                # then PMU = W2mu^T hT + b2mu. M=128, fp8 DoubleRow.
                pln = psO.tile([128, B], F32, name="pln")
                for j in range(8):
                    nc.tensor.matmul(pln[:], w2[:, 1, j, :, :],
                                     hT[:, 2 * j:2 * j + 2, :],
                                     start=(j == 0), stop=False, perf_mode=DR)
                nc.tensor.matmul(pln[:], bb[0:3, BB_LNW:BB_LNW + 128],
                                 bb[0:3, BB_LNR:BB_LNR + B],
                                 start=False, stop=True)
                pmu = psO.tile([128, B], F32, name="pmu")
                for j in range(8):
                    nc.tensor.matmul(pmu[:], w2[:, 0, j, :, :],
                                     hT[:, 2 * j:2 * j + 2, :],
                                     start=(j == 0), stop=False, perf_mode=DR)
                nc.tensor.matmul(pmu[:], bb[0:1, BB_MUW:BB_MUW + 128],
                                 bb[0:1, BB_MUR:BB_MUR + B],
                                 start=False, stop=True)

                # ---- epilogue: two column halves pipelined across
                # ACT/DVE/GpSimd. Sign trick: z*erf(z) and e^{-z^2} are
                # even, so we compute -z (saving the mu_x*inv op) and only
                # fix the sign of the standalone erf(z_127) term.
                HB = B // 2
                inv = work.tile([128, B], F32, name="inv")
                nc.scalar.activation(inv[:], pln[:], AF.Exp,
                                     bias=0.0, scale=-1.0)
                isp = work.tile([128, B], F32, name="isp")
                nc.scalar.activation(isp[:], pln[:], AF.Exp,
                                     bias=eps[:, EP_BLN2:EP_BLN2 + 1],
                                     scale=1.0)

                a4 = work.tile([128, B], F32, name="a4")
                md = work.tile([128, B], F32, name="md")
                zab = work.tile([128, 2, 3, HB], F32, name="zab")
                sq = work.tile([128, 2, 3, HB], F32, name="sq")
                cu = work.tile([128, 2, 3, HB], F32, name="cu")
                uu = work.tile([128, 2, 3, HB], F32, name="uu")
                erf = work.tile([128, 2, 3, HB], F32, name="erf")
                exg = work.tile([128, 2, 2, HB], F32, name="exg")
                a1 = work.tile([128, B], F32, name="a1")
                b1t = work.tile([128, B], F32, name="b1t")
                c1 = work.tile([128, B], F32, name="c1")
                d1 = work.tile([128, B], F32, name="d1")
                e1 = work.tile([128, B], F32, name="e1")
                sg = work.tile([128, B], F32, name="sg")
                gg = work.tile([128, B], F32, name="gg")
                hh = work.tile([128, B], F32, name="hh")
                hs = work.tile([128, B], F32, name="hs")
                h2 = work.tile([128, B], F32, name="h2")
                part = work.tile([128, 2], F32, name="part")

                def front(h):
                    bs = slice(h * HB, (h + 1) * HB)
                    nc.vector.tensor_tensor(
                        a4[:, bs], eps[:, EP_RM + h * HB:EP_RM + h * HB + HB],
                        pmu[:, bs], MULT)
                    nc.vector.tensor_tensor(
                        md[:, bs], eps[:, EP_S + h * HB:EP_S + h * HB + HB],
                        a4[:, bs], SUB)
                    # zab holds -z: (-z_a, -z_b, -z_7)
                    for slot, c, op in ((0, 0.9921875, ADD),
                                        (1, 0.9921875, SUB),
                                        (2, 0.984375, SUB)):
                        nc.vector.scalar_tensor_tensor(
                            zab[:, h, slot, :], md[:, bs], c, inv[:, bs],
                            op0=op, op1=MULT)

                def cubic(h):
                    nc.vector.tensor_tensor(cu[:, h], sq[:, h], zab[:, h],
                                            MULT)
                    nc.vector.scalar_tensor_tensor(uu[:, h], cu[:, h], ERFC,
                                                   zab[:, h], op0=MULT,
                                                   op1=ADD)

                def tail(h):
                    bs = slice(h * HB, (h + 1) * HB)
                    nc.vector.tensor_tensor(a1[:, bs], zab[:, h, 1, :],
                                            erf[:, h, 1, :], MULT)
                    nc.vector.tensor_tensor(d1[:, bs], a1[:, bs], b1t[:, bs],
                                            SUB)
                    nc.vector.tensor_tensor(e1[:, bs], d1[:, bs], c1[:, bs],
                                            ADD)
                    nc.vector.tensor_tensor(sg[:, bs], e1[:, bs], isp[:, bs],
                                            MULT)
                    nc.vector.tensor_tensor(
                        gg[:, bs], sg[:, bs],
                        eps[:, EP_XC + h * HB:EP_XC + h * HB + HB], ADD)
                    nc.vector.scalar_tensor_tensor(
                        hh[:, bs], erf[:, h, 2, :], -127.0 / 256.0, gg[:, bs],
                        op0=MULT, op1=SUB)
                    nc.vector.tensor_tensor(
                        hs[:, bs], hh[:, bs],
                        eps[:, EP_SQW + h * HB:EP_SQW + h * HB + HB], MULT)
                    nc.vector.scalar_tensor_tensor(
                        h2[:, bs], hs[:, bs], 1.0, hs[:, bs],
                        op0=BYP, op1=MULT, accum_out=part[:, h:h + 1])

                def gp_side(h):
                    bs = slice(h * HB, (h + 1) * HB)
                    nc.gpsimd.tensor_tensor(b1t[:, bs], zab[:, h, 0, :],
                                            erf[:, h, 0, :], MULT)
                    nc.gpsimd.tensor_tensor(c1[:, bs], exg[:, h, 1, :],
                                            exg[:, h, 0, :], SUB)

                front(0)
                front(1)
                nc.scalar.activation(sq[:, 0], zab[:, 0], AF.Square,
                                     bias=0.0, scale=1.0)
                cubic(0)
                nc.scalar.activation(erf[:, 0], uu[:, 0], AF.Tanh,
                                     bias=0.0, scale=ERFA)
                nc.scalar.activation(sq[:, 1], zab[:, 1], AF.Square,
                                     bias=0.0, scale=1.0)
                nc.scalar.activation(exg[:, 0], sq[:, 0, 0:2, :], AF.Exp,
                                     bias=eps[:, EP_BPI:EP_BPI + 1],
                                     scale=-1.0)
                cubic(1)
                nc.scalar.activation(erf[:, 1], uu[:, 1], AF.Tanh,
                                     bias=0.0, scale=ERFA)
                nc.scalar.activation(exg[:, 1], sq[:, 1, 0:2, :], AF.Exp,
                                     bias=eps[:, EP_BPI:EP_BPI + 1],
                                     scale=-1.0)
                gp_side(0)
                tail(0)
                gp_side(1)
                tail(1)

                # cross-partition reduce: [2,1] psum via ones-matmul, so
                # the output DMA is two 4-byte descriptors.
                ps1 = psO.tile([2, 1], F32, name="ps1")
                nc.tensor.matmul(ps1[:], part[:],
                                 eps[:, EP_ONE:EP_ONE + 1],
                                 start=True, stop=True)
                sres = work.tile([2, 1], F32, name="sres")
                nc.vector.tensor_copy(sres[:], ps1[:])
                nc.sync.dma_start(d_out.ap()[:], sres[:])

    nc.compile()
    return nc


def host_prep(x, t, noise, W1, b1, W2, b2):
    """Build the per-core in_maps (host-side packing + tiny per-row math)."""
    f32 = np.float32
    tv = t[:, 0].astype(f32)
    assert float(tv.min()) > 1e-8, "low-t mask path not implemented"
    gamma = (1.0 - np.power(f32(SIGMA1), f32(2.0) * tv)).astype(f32)
    assert float(gamma.min()) > 0.0
    r = np.sqrt((1.0 - gamma) / gamma).astype(f32)
    lnce = np.log(1.0 / (r * np.sqrt(f32(2.0)))).astype(f32)
    nlh = (-lnce).astype(BFNP)
    nll = ((-lnce) - nlh.astype(f32)).astype(BFNP)
    sqw = np.power(f32(SIGMA1), -tv).astype(f32)

    mu = (gamma[:, None] * x + (gamma * (1 - gamma))[:, None] * noise
          ).astype(f32)
    muT8 = np.ascontiguousarray(
        mu.T.reshape(8, 128, B).transpose(1, 0, 2).reshape(128, 8 * B)
        .astype(F8NP))

    # w1m[p, (m*8+k)*128 + c] = W1[k*128+p, m*128+c]
    w1f = W1[:D].astype(f32).reshape(8, 128, 16, 128)
    w1m = np.ascontiguousarray(
        w1f.transpose(1, 2, 0, 3).reshape(128, 16 * 8 * 128).astype(F8NP))

    bbv = np.zeros((4, BB_W), dtype=BFNP)
    bbv[0, BB_TV:BB_TV + B] = tv.astype(BFNP)
    bbv[1, BB_TV:BB_TV + B] = BFNP(1.0)
    bbv[0, BB_W1T:BB_W1T + H] = W1[D].astype(BFNP)
    bbv[1, BB_W1T:BB_W1T + H] = b1.astype(BFNP)
    bbv[1, BB_LNW:BB_LNW + 128] = BFNP(1.0)
    bbv[2, BB_LNW:BB_LNW + 128] = BFNP(1.0)
    bbv[0, BB_LNR:BB_LNR + B] = BFNP(1.0)
    bbv[1, BB_LNR:BB_LNR + B] = nlh
    bbv[2, BB_LNR:BB_LNR + B] = nll
    bbv[0, BB_MUR:BB_MUR + B] = BFNP(1.0)

    epv = np.zeros((128, EP_W), dtype=f32)
    epv[:, EP_RM:EP_RM + B] = r[None, :]
    epv[:, EP_SQW:EP_SQW + B] = sqw[None, :]
    epv[:, EP_ONE] = 1.0
    epv[:, EP_BLN2] = -LN2
    epv[:, EP_BPI] = -LNPI2

    S_full = (x + (1.0 - gamma)[:, None] * noise).astype(f32)
    XC_full = (x + f32(127.0 / 256.0)).astype(f32)

    in_maps = []
    for i in range(NCORES):
        dsl = slice(i * DSL, (i + 1) * DSL)
        # w2m[p, ((half*8+j)*2+rr)*128+c] = W2[(2j+rr)*128+p, half*D + col]
        w2c = np.stack([W2[:, dsl], W2[:, D + i * DSL:D + (i + 1) * DSL]],
                       axis=0)  # [2, 2048, 128]
        w2m = np.ascontiguousarray(
            w2c.reshape(2, 16, 128, 128).transpose(2, 0, 1, 3)
            .reshape(128, 2 * 16 * 128).astype(F8NP))
        bbi = bbv.copy()
        bbi[0, BB_LNW:BB_LNW + 128] = b2[D + i * DSL:D + (i + 1) * DSL
                                         ].astype(BFNP)
        bbi[0, BB_MUW:BB_MUW + 128] = b2[dsl].astype(BFNP)
        epi = epv.copy()
        epi[:, EP_S:EP_S + B] = S_full[:, dsl].T
        epi[:, EP_XC:EP_XC + B] = XC_full[:, dsl].T
        in_maps.append({
            "muT8": muT8, "w1m": w1m, "w2m": w2m, "bb": bbi, "ep": epi,
        })
    return in_maps


_nc_cache = {}


def get_nc(debug=False):
    if debug not in _nc_cache:
        _nc_cache[debug] = _build(debug)
    return _nc_cache[debug]


def run_on_cores(inputs, trace=False, debug=False, tmpdir=None):
    nc = get_nc(debug)
    in_maps = host_prep(**inputs)
    res = run_bass_kernel_spmd(nc, in_maps, core_ids=list(range(NCORES)),
                               trace=trace, tmpdir=tmpdir)
    total = np.float64(0.0)
    for i in range(NCORES):
        total += res.results[i]["outp"].astype(np.float64).sum()
    loss = np.float32(-np.log(np.float32(SIGMA1)) * total / float(B * D))
    return loss, res


_reset_done = [False]


def _maybe_reset_device():
    # Clear a wedged NRT exec unit left by a previous process. Best-effort.
    if _reset_done[0]:
        return
    _reset_done[0] = True
    try:
        import os
        import ctypes
        so = "/opt/axon/libaxon_pjrt.so"
        if os.path.exists(so):
            import jax

            jax.devices()
            lib = ctypes.CDLL(so)
            lib.axon_reset.restype = ctypes.c_int64
            lib.axon_reset()
    except Exception:
        pass


def kernel(**inputs):
    _maybe_reset_device()
    inputs = {k: np.asarray(v) for k, v in inputs.items()}
    loss, _ = run_on_cores(inputs)
    return np.asarray(loss, dtype=np.float32)


# revision 17
# speedup vs baseline: 1.9641x; 1.0608x over previous
"""Trainium2 Bass kernel for nn_DiscretisedBNF (discretised BNF loss).

Math: the reference's (B, D, K=128) clamped-CDF bin sum Abel-collapses to

    pO[b,d] = -127/256 - (1/128)*Sigma + (127/256)*erf(z_127),
    Sigma   = sum_{k=1..127} erf(z_k),  z_k = (e_k - mu_x)*inv

and Sigma is a uniform-grid Riemann sum of erf, so by Poisson summation
it equals the midpoint integral up to O(exp(-pi^2/s^2)) (s = inv/64):

    Sigma ~= (1/s)[ z_b*erf(z_b) - z_a*erf(z_a) + (e^{-z_b^2}-e^{-z_a^2})/sqrt(pi) ]
    z_a = inv*(-0.9921875) - mu_x*inv,  z_b = inv*(0.9921875) - mu_x*inv

This replaces the whole 127-bin binning phase (4.2M tanh + z/q matmuls
per core) with ~5 ACT passes and ~15 vector ops per [128,256] tile.
erf is evaluated as tanh((2/sqrt(pi))*(z + c*z^3)) (max abs err 3.6e-4),
so exp+tanh+square+prelu all live in the one resident ACT table set
(exp_and_others) -- no table switches.  End-to-end numpy mirror of the
device math (incl. fp8/bf16 quantization): rel err ~8e-5.

Constant foldings: mm2's ln-tile bias rows add -ln(cexp) (hi/lo bf16
split) so that  inv = exp(-PLN)  and  1/(128 s) = exp(PLN - ln2)  come
straight out of ACT with scalar biases; 1/sqrt(pi) is folded into the
exp bias.

Sharding (8 cores, full inputs in, full output out): mm1 replicated
(fp8 DoubleRow), W2 column-sharded 128+128 cols per core, epilogue
data-parallel on the core's [128 d x 256 b] tile. Output is a single
f32 partial per core (cross-partition reduce via a ones-matmul) so the
final DMA is one 4-byte descriptor. Host sums 8 partials.

PE warm-up: ~8 junk N=512 matmuls on a memset tile right at kernel
start keep HAM from running mm1 at the cold 1.2 GHz clock.
"""

import sys

sys.path.insert(0, "/opt/trn_rl_repo")

import numpy as np
import ml_dtypes

import concourse.bass as bass
import concourse.tile as tile
from concourse import bacc, mybir
from concourse.alu_op_type import AluOpType
from concourse.bass_utils import run_bass_kernel_spmd

B, D, H = 256, 1024, 2048
NCORES = 8
DSL = D // NCORES  # 128 d-columns per core
SIGMA1 = 0.02

F32 = mybir.dt.float32
BF16 = mybir.dt.bfloat16
FP8 = mybir.dt.float8e4
BFNP = ml_dtypes.bfloat16
F8NP = ml_dtypes.float8_e4m3

ERFA = float(2.0 / np.sqrt(np.pi))      # tanh scale
ERFC = float(0.10091075 / ERFA)          # z^3 coefficient (fit, err 3.6e-4)
LNPI2 = float(0.5 * np.log(np.pi))       # folded into exp(-z^2) bias
LN2 = float(np.log(2.0))

# bb blob (bf16, 4 partitions) column offsets
BB_TV = 0        # [0:2, 0:256]      row0 = t, row1 = ones (mm1 bias rhs)
BB_W1T = 256     # [0:2, 256:2304]   row0 = W1[D,:], row1 = b1 (mm1 bias lhsT)
BB_LNW = 2304    # [0:3, 2304:2432]  LN bias lhsT: [b2ln; 1; 1]
BB_LNR = 2432    # [0:3, 2432:2688]  LN bias rhs: [1; -lnCE_hi; -lnCE_lo]
BB_MUW = 2688    # [0:1, 2688:2816]  MU bias lhsT: b2mu
BB_MUR = 2816    # [0:1, 2816:3072]  MU bias rhs: ones
BB_W = 3072

# ep blob (f32, 128 partitions) column offsets
EP_S, EP_RM, EP_XC, EP_SQW, EP_ONE = 0, 256, 512, 768, 1024
EP_BLN2, EP_BPI = 1025, 1026   # bias columns: -ln2, -0.5*ln(pi)
EP_W = 1027


def _build(debug=False):
    nc = bacc.Bacc("TRN2", target_bir_lowering=False, debug=False,
                   num_devices=NCORES)

    d_muT = nc.dram_tensor("muT8", (128, 8 * B), FP8, kind="ExternalInput")
    d_w1 = nc.dram_tensor("w1m", (128, 16 * 8 * 128), FP8,
                          kind="ExternalInput")
    d_w2 = nc.dram_tensor("w2m", (128, 2 * 8 * 2 * 128), FP8,
                          kind="ExternalInput")
    d_bb = nc.dram_tensor("bb", (4, BB_W), BF16, kind="ExternalInput")
    d_ep = nc.dram_tensor("ep", (128, EP_W), F32, kind="ExternalInput")
    d_out = nc.dram_tensor("outp", (2, 1), F32, kind="ExternalOutput")

    MULT, ADD, SUB, BYP = (AluOpType.mult, AluOpType.add,
                           AluOpType.subtract, AluOpType.bypass)
    AF = mybir.ActivationFunctionType
    DR = mybir.MatmulPerfMode.DoubleRow

    with tile.TileContext(nc) as tc:
        with (
            tc.tile_pool(name="weights", bufs=1) as wpool,
            tc.tile_pool(name="work", bufs=1) as work,
        ):
            muT = wpool.tile([128, 8, B], FP8)
            w1s = [wpool.tile([128, 2, 8, 128], FP8, name=f"w1s{i}")
                   for i in range(8)]
            w2 = wpool.tile([128, 2, 8, 2, 128], FP8)
            bb = wpool.tile([4, BB_W], BF16)
            eps = wpool.tile([128, EP_W], F32)
            jw = wpool.tile([128, 640], BF16)
            hT = work.tile([128, 16, B], FP8)

            with (
                tc.tile_pool(name="psJ", bufs=1,
                             space=bass.MemorySpace.PSUM) as psJ,
                tc.tile_pool(name="psA", bufs=3,
                             space=bass.MemorySpace.PSUM) as psA,
                tc.tile_pool(name="psO", bufs=1,
                             space=bass.MemorySpace.PSUM) as psO,
            ):
                # ---- input DMAs: sync (HWDGE) carries the mm1-critical
                # tensors in need-order. The mm2/epilogue tensors go on
                # gpsimd (SWDGE) but are deferred behind mm1's first tile
                # (dummy dep below) so they don't steal DMA-engine
                # bandwidth from the mm1-critical transfers.
                nc.gpsimd.memset(jw[:], 0.0)
                nc.sync.dma_start(w1s[0][:], d_w1.ap()[:, 0:2048])
                nc.sync.dma_start(muT[:], d_muT.ap()[:])
                nc.sync.dma_start(bb[:], d_bb.ap()[:])
                for s4 in range(1, 8):
                    nc.sync.dma_start(
                        w1s[s4][:], d_w1.ap()[:, s4 * 2048:(s4 + 1) * 2048])
                # mm2/epilogue tensors at the sync-queue tail: their
                # transfers start only after the mm1-critical ones, so
                # they don't steal DMA-engine bandwidth from them.
                nc.sync.dma_start(w2[:], d_w2.ap()[:])
                nc.sync.dma_start(eps[:], d_ep.ap()[:])

                # ---- PE warm-up: dense junk matmuls (no DMA deps) so HAM
                # lifts the 1.2GHz cold clock before mm1's data lands.
                jp = psJ.tile([128, 512], F32)
                for _ in range(5):
                    nc.tensor.matmul(jp[:], jw[:, 0:128], jw[:, 128:640],
                                     start=True, stop=True)

                # ---- mm1: hT[m] = PRelu(W1^T mu_cat^T) fp8 DoubleRow;
                # t-row and b1 folded in as a K=2 bf16 matmul.
                for m in range(16):
                    ph = psA.tile([128, B], F32, tag="ph")
                    for j in range(4):
                        nc.tensor.matmul(
                            ph[:], w1s[m // 2][:, m % 2, 2 * j:2 * j + 2, :],
                            muT[:, 2 * j:2 * j + 2, :],
                            start=(j == 0), stop=False, perf_mode=DR)
                    ms = slice(BB_W1T + m * 128, BB_W1T + (m + 1) * 128)
                    nc.tensor.matmul(ph[:], bb[0:2, ms],
                                     bb[0:2, BB_TV:BB_TV + B],
                                     start=False, stop=True)
                    nc.scalar.activation(hT[:, m, :], ph[:], AF.Prelu,
                                         bias=0.0, scale=1.0, alpha=0.01)

                # ---- mm2: PMU = W2mu^T hT + b2mu first (so the a4/md
                # vector ops overlap the LN matmuls), then PLN = W2ln^T hT
                # + b2ln - lnCE (hi/lo bf16 rows). M=128, fp8 DoubleRow.
                pmu = psO.tile([128, B], F32, name="pmu")
                for j in range(8):
                    nc.tensor.matmul(pmu[:], w2[:, 0, j, :, :],
                                     hT[:, 2 * j:2 * j + 2, :],
                                     start=(j == 0), stop=False, perf_mode=DR)
                nc.tensor.matmul(pmu[:], bb[0:1, BB_MUW:BB_MUW + 128],
                                 bb[0:1, BB_MUR:BB_MUR + B],
                                 start=False, stop=True)
                pln = psO.tile([128, B], F32, name="pln")
                for j in range(8):
                    nc.tensor.matmul(pln[:], w2[:, 1, j, :, :],
                                     hT[:, 2 * j:2 * j + 2, :],
                                     start=(j == 0), stop=False, perf_mode=DR)
                nc.tensor.matmul(pln[:], bb[0:3, BB_LNW:BB_LNW + 128],
                                 bb[0:3, BB_LNR:BB_LNR + B],
                                 start=False, stop=True)

                # ---- epilogue: two column halves pipelined across
                # ACT/DVE/GpSimd. Sign trick: z*erf(z) and e^{-z^2} are
                # even, so we compute -z (saving the mu_x*inv op) and only
                # fix the sign of the standalone erf(z_127) term.
                HB = B // 2
                inv = work.tile([128, B], F32, name="inv")
                nc.scalar.activation(inv[:], pln[:], AF.Exp,
                                     bias=0.0, scale=-1.0)
                isp = work.tile([128, B], F32, name="isp")
                nc.scalar.activation(isp[:], pln[:], AF.Exp,
                                     bias=eps[:, EP_BLN2:EP_BLN2 + 1],
                                     scale=1.0)

                a4 = work.tile([128, B], F32, name="a4")
                md = work.tile([128, B], F32, name="md")
                zab = work.tile([128, 2, 3, HB], F32, name="zab")
                sq = work.tile([128, 2, 3, HB], F32, name="sq")
                cu = work.tile([128, 2, 3, HB], F32, name="cu")
                uu = work.tile([128, 2, 3, HB], F32, name="uu")
                erf = work.tile([128, 2, 3, HB], F32, name="erf")
                exg = work.tile([128, 2, 2, HB], F32, name="exg")
                a1 = work.tile([128, B], F32, name="a1")
                b1t = work.tile([128, B], F32, name="b1t")
                c1 = work.tile([128, B], F32, name="c1")
                d1 = work.tile([128, B], F32, name="d1")
                e1 = work.tile([128, B], F32, name="e1")
                sg = work.tile([128, B], F32, name="sg")
                gg = work.tile([128, B], F32, name="gg")
                hh = work.tile([128, B], F32, name="hh")
                hs = work.tile([128, B], F32, name="hs")
                h2 = work.tile([128, B], F32, name="h2")
                part = work.tile([128, 2], F32, name="part")

                def front_md(h):
                    bs = slice(h * HB, (h + 1) * HB)
                    nc.vector.tensor_tensor(
                        a4[:, bs], eps[:, EP_RM + h * HB:EP_RM + h * HB + HB],
                        pmu[:, bs], MULT)
                    nc.vector.tensor_tensor(
                        md[:, bs], eps[:, EP_S + h * HB:EP_S + h * HB + HB],
                        a4[:, bs], SUB)

                def front(h):
                    bs = slice(h * HB, (h + 1) * HB)
                    # zab holds -z: (-z_a, -z_b, -z_7)
                    for slot, c, op in ((0, 0.9921875, ADD),
                                        (1, 0.9921875, SUB),
                                        (2, 0.984375, SUB)):
                        nc.vector.scalar_tensor_tensor(
                            zab[:, h, slot, :], md[:, bs], c, inv[:, bs],
                            op0=op, op1=MULT)

                def cubic(h):
                    nc.vector.tensor_tensor(cu[:, h], sq[:, h], zab[:, h],
                                            MULT)
                    nc.vector.scalar_tensor_tensor(uu[:, h], cu[:, h], ERFC,
                                                   zab[:, h], op0=MULT,
                                                   op1=ADD)

                def tail(h):
                    bs = slice(h * HB, (h + 1) * HB)
                    nc.vector.tensor_tensor(a1[:, bs], zab[:, h, 1, :],
                                            erf[:, h, 1, :], MULT)
                    nc.vector.tensor_tensor(d1[:, bs], a1[:, bs], b1t[:, bs],
                                            SUB)
                    nc.vector.tensor_tensor(e1[:, bs], d1[:, bs], c1[:, bs],
                                            ADD)
                    nc.vector.tensor_tensor(sg[:, bs], e1[:, bs], isp[:, bs],
                                            MULT)
                    nc.vector.tensor_tensor(
                        gg[:, bs], sg[:, bs],
                        eps[:, EP_XC + h * HB:EP_XC + h * HB + HB], ADD)
                    nc.vector.scalar_tensor_tensor(
                        hh[:, bs], erf[:, h, 2, :], -127.0 / 256.0, gg[:, bs],
                        op0=MULT, op1=SUB)
                    nc.vector.tensor_tensor(
                        hs[:, bs], hh[:, bs],
                        eps[:, EP_SQW + h * HB:EP_SQW + h * HB + HB], MULT)
                    nc.vector.scalar_tensor_tensor(
                        h2[:, bs], hs[:, bs], 1.0, hs[:, bs],
                        op0=BYP, op1=MULT, accum_out=part[:, h:h + 1])

                def gp_side(h):
                    bs = slice(h * HB, (h + 1) * HB)
                    nc.gpsimd.tensor_tensor(b1t[:, bs], zab[:, h, 0, :],
                                            erf[:, h, 0, :], MULT)
                    nc.gpsimd.tensor_tensor(c1[:, bs], exg[:, h, 1, :],
                                            exg[:, h, 0, :], SUB)

                front_md(0)
                front_md(1)
                front(0)
                front(1)
                nc.scalar.activation(sq[:, 0], zab[:, 0], AF.Square,
                                     bias=0.0, scale=1.0)
                cubic(0)
                nc.scalar.activation(erf[:, 0], uu[:, 0], AF.Tanh,
                                     bias=0.0, scale=ERFA)
                nc.scalar.activation(sq[:, 1], zab[:, 1], AF.Square,
                                     bias=0.0, scale=1.0)
                nc.scalar.activation(exg[:, 0], sq[:, 0, 0:2, :], AF.Exp,
                                     bias=eps[:, EP_BPI:EP_BPI + 1],
                                     scale=-1.0)
                cubic(1)
                nc.scalar.activation(erf[:, 1], uu[:, 1], AF.Tanh,
                                     bias=0.0, scale=ERFA)
                nc.scalar.activation(exg[:, 1], sq[:, 1, 0:2, :], AF.Exp,
                                     bias=eps[:, EP_BPI:EP_BPI + 1],
                                     scale=-1.0)
                gp_side(0)
                tail(0)
                gp_side(1)
                tail(1)

                # cross-partition reduce: [2,1] psum via ones-matmul, so
                # the output DMA is two 4-byte descriptors.
                ps1 = psO.tile([2, 1], F32, name="ps1")
                nc.tensor.matmul(ps1[:], part[:],
                                 eps[:, EP_ONE:EP_ONE + 1],
                                 start=True, stop=True)
                sres = work.tile([2, 1], F32, name="sres")
                nc.vector.tensor_copy(sres[:], ps1[:])
                nc.sync.dma_start(d_out.ap()[:], sres[:])

    nc.compile()
    return nc


def host_prep(x, t, noise, W1, b1, W2, b2):
    """Build the per-core in_maps (host-side packing + tiny per-row math)."""
    f32 = np.float32
    tv = t[:, 0].astype(f32)
    assert float(tv.min()) > 1e-8, "low-t mask path not implemented"
    gamma = (1.0 - np.power(f32(SIGMA1), f32(2.0) * tv)).astype(f32)
    assert float(gamma.min()) > 0.0
    r = np.sqrt((1.0 - gamma) / gamma).astype(f32)
    lnce = np.log(1.0 / (r * np.sqrt(f32(2.0)))).astype(f32)
    nlh = (-lnce).astype(BFNP)
    nll = ((-lnce) - nlh.astype(f32)).astype(BFNP)
    sqw = np.power(f32(SIGMA1), -tv).astype(f32)

    mu = (gamma[:, None] * x + (gamma * (1 - gamma))[:, None] * noise
          ).astype(f32)
    muT8 = np.ascontiguousarray(
        mu.T.reshape(8, 128, B).transpose(1, 0, 2).reshape(128, 8 * B)
        .astype(F8NP))

    # w1m[p, (m*8+k)*128 + c] = W1[k*128+p, m*128+c]
    w1f = W1[:D].astype(f32).reshape(8, 128, 16, 128)
    w1m = np.ascontiguousarray(
        w1f.transpose(1, 2, 0, 3).reshape(128, 16 * 8 * 128).astype(F8NP))

    bbv = np.zeros((4, BB_W), dtype=BFNP)
    bbv[0, BB_TV:BB_TV + B] = tv.astype(BFNP)
    bbv[1, BB_TV:BB_TV + B] = BFNP(1.0)
    bbv[0, BB_W1T:BB_W1T + H] = W1[D].astype(BFNP)
    bbv[1, BB_W1T:BB_W1T + H] = b1.astype(BFNP)
    bbv[1, BB_LNW:BB_LNW + 128] = BFNP(1.0)
    bbv[2, BB_LNW:BB_LNW + 128] = BFNP(1.0)
    bbv[0, BB_LNR:BB_LNR + B] = BFNP(1.0)
    bbv[1, BB_LNR:BB_LNR + B] = nlh
    bbv[2, BB_LNR:BB_LNR + B] = nll
    bbv[0, BB_MUR:BB_MUR + B] = BFNP(1.0)

    epv = np.zeros((128, EP_W), dtype=f32)
    epv[:, EP_RM:EP_RM + B] = r[None, :]
    epv[:, EP_SQW:EP_SQW + B] = sqw[None, :]
    epv[:, EP_ONE] = 1.0
    epv[:, EP_BLN2] = -LN2
    epv[:, EP_BPI] = -LNPI2

    S_full = (x + (1.0 - gamma)[:, None] * noise).astype(f32)
    XC_full = (x + f32(127.0 / 256.0)).astype(f32)

    in_maps = []
    for i in range(NCORES):
        dsl = slice(i * DSL, (i + 1) * DSL)
        # w2m[p, ((half*8+j)*2+rr)*128+c] = W2[(2j+rr)*128+p, half*D + col]
        w2c = np.stack([W2[:, dsl], W2[:, D + i * DSL:D + (i + 1) * DSL]],
                       axis=0)  # [2, 2048, 128]
        w2m = np.ascontiguousarray(
            w2c.reshape(2, 16, 128, 128).transpose(2, 0, 1, 3)
            .reshape(128, 2 * 16 * 128).astype(F8NP))
        bbi = bbv.copy()
        bbi[0, BB_LNW:BB_LNW + 128] = b2[D + i * DSL:D + (i + 1) * DSL
                                         ].astype(BFNP)
        bbi[0, BB_MUW:BB_MUW + 128] = b2[dsl].astype(BFNP)
        epi = epv.copy()
        epi[:, EP_S:EP_S + B] = S_full[:, dsl].T
        epi[:, EP_XC:EP_XC + B] = XC_full[:, dsl].T
        in_maps.append({
            "muT8": muT8, "w1m": w1m, "w2m": w2m, "bb": bbi, "ep": epi,
        })
    return in_maps


_nc_cache = {}


def get_nc(debug=False):
    if debug not in _nc_cache:
        _nc_cache[debug] = _build(debug)
    return _nc_cache[debug]


def run_on_cores(inputs, trace=False, debug=False, tmpdir=None):
    nc = get_nc(debug)
    in_maps = host_prep(**inputs)
    res = run_bass_kernel_spmd(nc, in_maps, core_ids=list(range(NCORES)),
                               trace=trace, tmpdir=tmpdir)
    total = np.float64(0.0)
    for i in range(NCORES):
        total += res.results[i]["outp"].astype(np.float64).sum()
    loss = np.float32(-np.log(np.float32(SIGMA1)) * total / float(B * D))
    return loss, res


_reset_done = [False]


def _maybe_reset_device():
    # Clear a wedged NRT exec unit left by a previous process. Best-effort.
    if _reset_done[0]:
        return
    _reset_done[0] = True
    try:
        import os
        import ctypes
        so = "/opt/axon/libaxon_pjrt.so"
        if os.path.exists(so):
            import jax

            jax.devices()
            lib = ctypes.CDLL(so)
            lib.axon_reset.restype = ctypes.c_int64
            lib.axon_reset()
    except Exception:
        pass


def kernel(**inputs):
    _maybe_reset_device()
    inputs = {k: np.asarray(v) for k, v in inputs.items()}
    loss, _ = run_on_cores(inputs)
    return np.asarray(loss, dtype=np.float32)


# revision 19
# speedup vs baseline: 2.1014x; 1.0699x over previous
"""Trainium2 Bass kernel for nn_DiscretisedBNF (discretised BNF loss).

Math: the reference's (B, D, K=128) clamped-CDF bin sum Abel-collapses to

    pO[b,d] = -127/256 - (1/128)*Sigma + (127/256)*erf(z_127),
    Sigma   = sum_{k=1..127} erf(z_k),  z_k = (e_k - mu_x)*inv

and Sigma is a uniform-grid Riemann sum of erf, so by Poisson summation
it equals the midpoint integral up to O(exp(-pi^2/s^2)) (s = inv/64):

    Sigma ~= (1/s)[ z_b*erf(z_b) - z_a*erf(z_a) + (e^{-z_b^2}-e^{-z_a^2})/sqrt(pi) ]
    z_a = inv*(-0.9921875) - mu_x*inv,  z_b = inv*(0.9921875) - mu_x*inv

This replaces the whole 127-bin binning phase (4.2M tanh + z/q matmuls
per core) with ~5 ACT passes and ~15 vector ops per [128,256] tile.
erf is evaluated as tanh((2/sqrt(pi))*(z + c*z^3)) (max abs err 3.6e-4),
so exp+tanh+square+prelu all live in the one resident ACT table set
(exp_and_others) -- no table switches.  End-to-end numpy mirror of the
device math (incl. fp8/bf16 quantization): rel err ~8e-5.

Constant foldings: mm2's ln-tile bias rows add -ln(cexp) (hi/lo bf16
split) so that  inv = exp(-PLN)  and  1/(128 s) = exp(PLN - ln2)  come
straight out of ACT with scalar biases; 1/sqrt(pi) is folded into the
exp bias.

Sharding (8 cores, full inputs in, full output out): mm1 replicated
(fp8 DoubleRow), W2 column-sharded 128+128 cols per core, epilogue
data-parallel on the core's [128 d x 256 b] tile. Output is a single
f32 partial per core (cross-partition reduce via a ones-matmul) so the
final DMA is one 4-byte descriptor. Host sums 8 partials.

PE warm-up: ~8 junk N=512 matmuls on a memset tile right at kernel
start keep HAM from running mm1 at the cold 1.2 GHz clock.
"""

import sys

sys.path.insert(0, "/opt/trn_rl_repo")

import numpy as np
import ml_dtypes

import concourse.bass as bass
import concourse.tile as tile
from concourse import bacc, mybir
from concourse.alu_op_type import AluOpType
from concourse.bass_utils import run_bass_kernel_spmd

B, D, H = 256, 1024, 2048
NCORES = 8
DSL = D // NCORES  # 128 d-columns per core
SIGMA1 = 0.02

F32 = mybir.dt.float32
BF16 = mybir.dt.bfloat16
FP8 = mybir.dt.float8e4
BFNP = ml_dtypes.bfloat16
F8NP = ml_dtypes.float8_e4m3

ERFA = float(2.0 / np.sqrt(np.pi))      # tanh scale
ERFC = float(0.10091075 / ERFA)          # z^3 coefficient (fit, err 3.6e-4)
LNPI2 = float(0.5 * np.log(np.pi))       # folded into exp(-z^2) bias
LN2 = float(np.log(2.0))

# bb blob (bf16, 4 partitions) column offsets
BB_TV = 0        # [0:2, 0:256]      row0 = t, row1 = ones (mm1 bias rhs)
BB_W1T = 256     # [0:2, 256:2304]   row0 = W1[D,:], row1 = b1 (mm1 bias lhsT)
BB_LNW = 2304    # [0:3, 2304:2432]  LN bias lhsT: [b2ln; 1; 1]
BB_LNR = 2432    # [0:3, 2432:2688]  LN bias rhs: [1; -lnCE_hi; -lnCE_lo]
BB_MUW = 2688    # [0:1, 2688:2816]  MU bias lhsT: b2mu
BB_MUR = 2816    # [0:1, 2816:3072]  MU bias rhs: ones
BB_W = 3072

# ep blob (f32, 128 partitions) column offsets
EP_S, EP_RM, EP_XC, EP_SQW, EP_ONE = 0, 256, 512, 768, 1024
EP_BLN2, EP_BPI = 1025, 1026   # bias columns: -ln2, -0.5*ln(pi)
EP_W = 1027


def _build(debug=False):
    nc = bacc.Bacc("TRN2", target_bir_lowering=False, debug=False,
                   num_devices=NCORES)

    d_muT = nc.dram_tensor("muT8", (128, 8 * B), FP8, kind="ExternalInput")
    d_w1 = nc.dram_tensor("w1m", (128, 16 * 8 * 128), FP8,
                          kind="ExternalInput")
    d_w2 = nc.dram_tensor("w2m", (128, 2 * 8 * 2 * 128), FP8,
                          kind="ExternalInput")
    d_bb = nc.dram_tensor("bb", (4, BB_W), BF16, kind="ExternalInput")
    d_ep = nc.dram_tensor("ep", (128, EP_W), F32, kind="ExternalInput")
    d_out = nc.dram_tensor("outp", (2, 1), F32, kind="ExternalOutput")

    MULT, ADD, SUB, BYP = (AluOpType.mult, AluOpType.add,
                           AluOpType.subtract, AluOpType.bypass)
    AF = mybir.ActivationFunctionType
    DR = mybir.MatmulPerfMode.DoubleRow

    with tile.TileContext(nc) as tc:
        with (
            tc.tile_pool(name="weights", bufs=1) as wpool,
            tc.tile_pool(name="work", bufs=1) as work,
        ):
            muT = wpool.tile([128, 8, B], FP8)
            w1s = [wpool.tile([128, 2, 8, 128], FP8, name=f"w1s{i}")
                   for i in range(8)]
            w2 = wpool.tile([128, 2, 8, 2, 128], FP8)
            bb = wpool.tile([4, BB_W], BF16)
            eps = wpool.tile([128, EP_W], F32)
            jw = wpool.tile([128, 640], BF16)
            hT = work.tile([128, 16, B], FP8)

            with (
                tc.tile_pool(name="psJ", bufs=1,
                             space=bass.MemorySpace.PSUM) as psJ,
                tc.tile_pool(name="psA", bufs=4,
                             space=bass.MemorySpace.PSUM) as psA,
                tc.tile_pool(name="psO", bufs=1,
                             space=bass.MemorySpace.PSUM) as psO,
            ):
                # ---- input DMAs: sync (HWDGE) carries the mm1-critical
                # tensors in need-order. The mm2/epilogue tensors go on
                # gpsimd (SWDGE) but are deferred behind mm1's first tile
                # (dummy dep below) so they don't steal DMA-engine
                # bandwidth from the mm1-critical transfers.
                nc.gpsimd.memset(jw[:], 0.0)
                nc.sync.dma_start(w1s[0][:], d_w1.ap()[:, 0:2048])
                nc.sync.dma_start(muT[:], d_muT.ap()[:])
                nc.sync.dma_start(bb[:], d_bb.ap()[:])
                for s4 in range(1, 8):
                    nc.sync.dma_start(
                        w1s[s4][:], d_w1.ap()[:, s4 * 2048:(s4 + 1) * 2048])
                # mm2/epilogue tensors at the sync-queue tail: their
                # transfers start only after the mm1-critical ones, so
                # they don't steal DMA-engine bandwidth from them.
                nc.sync.dma_start(w2[:], d_w2.ap()[:])
                nc.sync.dma_start(eps[:], d_ep.ap()[:])

                # ---- PE warm-up: dense junk matmuls (no DMA deps) so HAM
                # lifts the 1.2GHz cold clock before mm1's data lands.
                jp = psJ.tile([128, 512], F32)
                for _ in range(12):
                    nc.tensor.matmul(jp[:], jw[:, 0:128], jw[:, 128:640],
                                     start=True, stop=True)

                # ---- mm1: hT[m] = PRelu(W1^T mu_cat^T) fp8 DoubleRow;
                # t-row and b1 folded in as a K=2 bf16 matmul.
                for m in range(16):
                    ph = psA.tile([128, B], F32, tag="ph")
                    for j in range(4):
                        nc.tensor.matmul(
                            ph[:], w1s[m // 2][:, m % 2, 2 * j:2 * j + 2, :],
                            muT[:, 2 * j:2 * j + 2, :],
                            start=(j == 0), stop=False, perf_mode=DR)
                    ms = slice(BB_W1T + m * 128, BB_W1T + (m + 1) * 128)
                    nc.tensor.matmul(ph[:], bb[0:2, ms],
                                     bb[0:2, BB_TV:BB_TV + B],
                                     start=False, stop=True)
                    nc.scalar.activation(hT[:, m, :], ph[:], AF.Prelu,
                                         bias=0.0, scale=1.0, alpha=0.01)

                # ---- mm2: PMU = W2mu^T hT + b2mu first (so the a4/md
                # vector ops overlap the LN matmuls), then PLN = W2ln^T hT
                # + b2ln - lnCE (hi/lo bf16 rows). M=128, fp8 DoubleRow.
                pmu = psO.tile([128, B], F32, name="pmu")
                for j in range(8):
                    nc.tensor.matmul(pmu[:], w2[:, 0, j, :, :],
                                     hT[:, 2 * j:2 * j + 2, :],
                                     start=(j == 0), stop=False, perf_mode=DR)
                nc.tensor.matmul(pmu[:], bb[0:1, BB_MUW:BB_MUW + 128],
                                 bb[0:1, BB_MUR:BB_MUR + B],
                                 start=False, stop=True)
                pln = psO.tile([128, B], F32, name="pln")
                for j in range(8):
                    nc.tensor.matmul(pln[:], w2[:, 1, j, :, :],
                                     hT[:, 2 * j:2 * j + 2, :],
                                     start=(j == 0), stop=False, perf_mode=DR)
                nc.tensor.matmul(pln[:], bb[0:3, BB_LNW:BB_LNW + 128],
                                 bb[0:3, BB_LNR:BB_LNR + B],
                                 start=False, stop=True)

                # ---- epilogue: two column halves pipelined across
                # ACT/DVE/GpSimd. Sign trick: z*erf(z) and e^{-z^2} are
                # even, so we compute -z (saving the mu_x*inv op) and only
                # fix the sign of the standalone erf(z_127) term.
                HB = B // 2
                inv = work.tile([128, B], F32, name="inv")
                nc.scalar.activation(inv[:], pln[:], AF.Exp,
                                     bias=0.0, scale=-1.0)
                isp = work.tile([128, B], F32, name="isp")
                nc.scalar.activation(isp[:], pln[:], AF.Exp,
                                     bias=eps[:, EP_BLN2:EP_BLN2 + 1],
                                     scale=1.0)

                a4 = work.tile([128, B], F32, name="a4")
                md = work.tile([128, B], F32, name="md")
                zab = work.tile([128, 2, 3, HB], F32, name="zab")
                sq = work.tile([128, 2, 3, HB], F32, name="sq")
                cu = work.tile([128, 2, 3, HB], F32, name="cu")
                uu = work.tile([128, 2, 3, HB], F32, name="uu")
                erf = work.tile([128, 2, 3, HB], F32, name="erf")
                exg = work.tile([128, 2, 2, HB], F32, name="exg")
                a1 = work.tile([128, B], F32, name="a1")
                b1t = work.tile([128, B], F32, name="b1t")
                c1 = work.tile([128, B], F32, name="c1")
                d1 = work.tile([128, B], F32, name="d1")
                e1 = work.tile([128, B], F32, name="e1")
                sg = work.tile([128, B], F32, name="sg")
                gg = work.tile([128, B], F32, name="gg")
                hh = work.tile([128, B], F32, name="hh")
                hs = work.tile([128, B], F32, name="hs")
                h2 = work.tile([128, B], F32, name="h2")
                part = work.tile([128, 2], F32, name="part")

                def front_md(h):
                    bs = slice(h * HB, (h + 1) * HB)
                    nc.vector.tensor_tensor(
                        a4[:, bs], eps[:, EP_RM + h * HB:EP_RM + h * HB + HB],
                        pmu[:, bs], MULT)
                    nc.vector.tensor_tensor(
                        md[:, bs], eps[:, EP_S + h * HB:EP_S + h * HB + HB],
                        a4[:, bs], SUB)

                def front(h):
                    bs = slice(h * HB, (h + 1) * HB)
                    # zab holds -z: (-z_a, -z_b, -z_7)
                    for slot, c, op in ((0, 0.9921875, ADD),
                                        (1, 0.9921875, SUB),
                                        (2, 0.984375, SUB)):
                        nc.vector.scalar_tensor_tensor(
                            zab[:, h, slot, :], md[:, bs], c, inv[:, bs],
                            op0=op, op1=MULT)

                def cubic(h):
                    nc.vector.tensor_tensor(cu[:, h], sq[:, h], zab[:, h],
                                            MULT)
                    nc.vector.scalar_tensor_tensor(uu[:, h], cu[:, h], ERFC,
                                                   zab[:, h], op0=MULT,
                                                   op1=ADD)

                def tail(h):
                    bs = slice(h * HB, (h + 1) * HB)
                    nc.vector.tensor_tensor(a1[:, bs], zab[:, h, 1, :],
                                            erf[:, h, 1, :], MULT)
                    nc.vector.tensor_tensor(d1[:, bs], a1[:, bs], b1t[:, bs],
                                            SUB)
                    nc.vector.tensor_tensor(e1[:, bs], d1[:, bs], c1[:, bs],
                                            ADD)
                    nc.vector.tensor_tensor(sg[:, bs], e1[:, bs], isp[:, bs],
                                            MULT)
                    nc.vector.tensor_tensor(
                        gg[:, bs], sg[:, bs],
                        eps[:, EP_XC + h * HB:EP_XC + h * HB + HB], ADD)
                    nc.vector.scalar_tensor_tensor(
                        hh[:, bs], erf[:, h, 2, :], -127.0 / 256.0, gg[:, bs],
                        op0=MULT, op1=SUB)
                    nc.vector.tensor_tensor(
                        hs[:, bs], hh[:, bs],
                        eps[:, EP_SQW + h * HB:EP_SQW + h * HB + HB], MULT)
                    nc.vector.scalar_tensor_tensor(
                        h2[:, bs], hs[:, bs], 1.0, hs[:, bs],
                        op0=BYP, op1=MULT, accum_out=part[:, h:h + 1])

                def gp_side(h):
                    bs = slice(h * HB, (h + 1) * HB)
                    nc.gpsimd.tensor_tensor(b1t[:, bs], zab[:, h, 0, :],
                                            erf[:, h, 0, :], MULT)
                    nc.gpsimd.tensor_tensor(c1[:, bs], exg[:, h, 1, :],
                                            exg[:, h, 0, :], SUB)

                front_md(0)
                front_md(1)
                front(0)
                front(1)
                nc.scalar.activation(sq[:, 0], zab[:, 0], AF.Square,
                                     bias=0.0, scale=1.0)
                cubic(0)
                nc.scalar.activation(erf[:, 0], uu[:, 0], AF.Tanh,
                                     bias=0.0, scale=ERFA)
                nc.scalar.activation(sq[:, 1], zab[:, 1], AF.Square,
                                     bias=0.0, scale=1.0)
                nc.scalar.activation(exg[:, 0], sq[:, 0, 0:2, :], AF.Exp,
                                     bias=eps[:, EP_BPI:EP_BPI + 1],
                                     scale=-1.0)
                cubic(1)
                nc.scalar.activation(erf[:, 1], uu[:, 1], AF.Tanh,
                                     bias=0.0, scale=ERFA)
                nc.scalar.activation(exg[:, 1], sq[:, 1, 0:2, :], AF.Exp,
                                     bias=eps[:, EP_BPI:EP_BPI + 1],
                                     scale=-1.0)
                gp_side(0)
                tail(0)
                gp_side(1)
                tail(1)

                # cross-partition reduce: [2,1] psum via ones-matmul, so
                # the output DMA is two 4-byte descriptors.
                ps1 = psO.tile([2, 1], F32, name="ps1")
                nc.tensor.matmul(ps1[:], part[:],
                                 eps[:, EP_ONE:EP_ONE + 1],
                                 start=True, stop=True)
                sres = work.tile([2, 1], F32, name="sres")
                nc.vector.tensor_copy(sres[:], ps1[:])
                nc.sync.dma_start(d_out.ap()[:], sres[:])

    nc.compile()
    return nc


def host_prep(x, t, noise, W1, b1, W2, b2):
    """Build the per-core in_maps (host-side packing + tiny per-row math)."""
    f32 = np.float32
    tv = t[:, 0].astype(f32)
    assert float(tv.min()) > 1e-8, "low-t mask path not implemented"
    gamma = (1.0 - np.power(f32(SIGMA1), f32(2.0) * tv)).astype(f32)
    assert float(gamma.min()) > 0.0
    r = np.sqrt((1.0 - gamma) / gamma).astype(f32)
    lnce = np.log(1.0 / (r * np.sqrt(f32(2.0)))).astype(f32)
    nlh = (-lnce).astype(BFNP)
    nll = ((-lnce) - nlh.astype(f32)).astype(BFNP)
    sqw = np.power(f32(SIGMA1), -tv).astype(f32)

    mu = (gamma[:, None] * x + (gamma * (1 - gamma))[:, None] * noise
          ).astype(f32)
    muT8 = np.ascontiguousarray(
        mu.T.reshape(8, 128, B).transpose(1, 0, 2).reshape(128, 8 * B)
        .astype(F8NP))

    # w1m[p, (m*8+k)*128 + c] = W1[k*128+p, m*128+c]
    w1f = W1[:D].astype(f32).reshape(8, 128, 16, 128)
    w1m = np.ascontiguousarray(
        w1f.transpose(1, 2, 0, 3).reshape(128, 16 * 8 * 128).astype(F8NP))

    bbv = np.zeros((4, BB_W), dtype=BFNP)
    bbv[0, BB_TV:BB_TV + B] = tv.astype(BFNP)
    bbv[1, BB_TV:BB_TV + B] = BFNP(1.0)
    bbv[0, BB_W1T:BB_W1T + H] = W1[D].astype(BFNP)
    bbv[1, BB_W1T:BB_W1T + H] = b1.astype(BFNP)
    bbv[1, BB_LNW:BB_LNW + 128] = BFNP(1.0)
    bbv[2, BB_LNW:BB_LNW + 128] = BFNP(1.0)
    bbv[0, BB_LNR:BB_LNR + B] = BFNP(1.0)
    bbv[1, BB_LNR:BB_LNR + B] = nlh
    bbv[2, BB_LNR:BB_LNR + B] = nll
    bbv[0, BB_MUR:BB_MUR + B] = BFNP(1.0)

    epv = np.zeros((128, EP_W), dtype=f32)
    epv[:, EP_RM:EP_RM + B] = r[None, :]
    epv[:, EP_SQW:EP_SQW + B] = sqw[None, :]
    epv[:, EP_ONE] = 1.0
    epv[:, EP_BLN2] = -LN2
    epv[:, EP_BPI] = -LNPI2

    S_full = (x + (1.0 - gamma)[:, None] * noise).astype(f32)
    XC_full = (x + f32(127.0 / 256.0)).astype(f32)

    in_maps = []
    for i in range(NCORES):
        dsl = slice(i * DSL, (i + 1) * DSL)
        # w2m[p, ((half*8+j)*2+rr)*128+c] = W2[(2j+rr)*128+p, half*D + col]
        w2c = np.stack([W2[:, dsl], W2[:, D + i * DSL:D + (i + 1) * DSL]],
                       axis=0)  # [2, 2048, 128]
        w2m = np.ascontiguousarray(
            w2c.reshape(2, 16, 128, 128).transpose(2, 0, 1, 3)
            .reshape(128, 2 * 16 * 128).astype(F8NP))
        bbi = bbv.copy()
        bbi[0, BB_LNW:BB_LNW + 128] = b2[D + i * DSL:D + (i + 1) * DSL
                                         ].astype(BFNP)
        bbi[0, BB_MUW:BB_MUW + 128] = b2[dsl].astype(BFNP)
        epi = epv.copy()
        epi[:, EP_S:EP_S + B] = S_full[:, dsl].T
        epi[:, EP_XC:EP_XC + B] = XC_full[:, dsl].T
        in_maps.append({
            "muT8": muT8, "w1m": w1m, "w2m": w2m, "bb": bbi, "ep": epi,
        })
    return in_maps


_nc_cache = {}


def get_nc(debug=False):
    if debug not in _nc_cache:
        _nc_cache[debug] = _build(debug)
    return _nc_cache[debug]


def run_on_cores(inputs, trace=False, debug=False, tmpdir=None):
    nc = get_nc(debug)
    in_maps = host_prep(**inputs)
    res = run_bass_kernel_spmd(nc, in_maps, core_ids=list(range(NCORES)),
                               trace=trace, tmpdir=tmpdir)
    total = np.float64(0.0)
    for i in range(NCORES):
        total += res.results[i]["outp"].astype(np.float64).sum()
    loss = np.float32(-np.log(np.float32(SIGMA1)) * total / float(B * D))
    return loss, res


_reset_done = [False]


def _maybe_reset_device():
    # Clear a wedged NRT exec unit left by a previous process. Best-effort.
    if _reset_done[0]:
        return
    _reset_done[0] = True
    try:
        import os
        import ctypes
        so = "/opt/axon/libaxon_pjrt.so"
        if os.path.exists(so):
            import jax

            jax.devices()
            lib = ctypes.CDLL(so)
            lib.axon_reset.restype = ctypes.c_int64
            lib.axon_reset()
    except Exception:
        pass


def kernel(**inputs):
    _maybe_reset_device()
    inputs = {k: np.asarray(v) for k, v in inputs.items()}
    loss, _ = run_on_cores(inputs)
    return np.asarray(loss, dtype=np.float32)


# revision 30
# speedup vs baseline: 2.3652x; 1.1255x over previous
"""Trainium2 Bass kernel for nn_DiscretisedBNF (discretised BNF loss).

Math: the reference's (B, D, K=128) clamped-CDF bin sum Abel-collapses to

    pO[b,d] = -127/256 - (1/128)*Sigma + (127/256)*erf(z_127),
    Sigma   = sum_{k=1..127} erf(z_k),  z_k = (e_k - mu_x)*inv

and Sigma is a uniform-grid Riemann sum of erf, so by Poisson summation
it equals the midpoint integral up to O(exp(-pi^2/s^2)) (s = inv/64):

    Sigma ~= (1/s)[ z_b*erf(z_b) - z_a*erf(z_a) + (e^{-z_b^2}-e^{-z_a^2})/sqrt(pi) ]
    z_a = inv*(-0.9921875) - mu_x*inv,  z_b = inv*(0.9921875) - mu_x*inv

This replaces the whole 127-bin binning phase (4.2M tanh + z/q matmuls
per core) with ~5 ACT passes and ~15 vector ops per [128,256] tile.
erf is evaluated as tanh((2/sqrt(pi))*(z + c*z^3)) (max abs err 3.6e-4),
so exp+tanh+square+prelu all live in the one resident ACT table set
(exp_and_others) -- no table switches.  End-to-end numpy mirror of the
device math (incl. fp8/bf16 quantization): rel err ~8e-5.

Constant foldings: mm2's ln-tile bias rows add -ln(cexp) (hi/lo bf16
split) so that  inv = exp(-PLN)  and  1/(128 s) = exp(PLN - ln2)  come
straight out of ACT with scalar biases; 1/sqrt(pi) is folded into the
exp bias.

Sharding (8 cores, full inputs in, full output out): mm1 replicated
(fp8 DoubleRow), W2 column-sharded 128+128 cols per core, epilogue
data-parallel on the core's [128 d x 256 b] tile. Output is a single
f32 partial per core (cross-partition reduce via a ones-matmul) so the
final DMA is one 4-byte descriptor. Host sums 8 partials.

PE warm-up: ~8 junk N=512 matmuls on a memset tile right at kernel
start keep HAM from running mm1 at the cold 1.2 GHz clock.
"""

import sys

sys.path.insert(0, "/opt/trn_rl_repo")

import numpy as np
import ml_dtypes

import concourse.bass as bass
import concourse.tile as tile
from concourse import bacc, mybir
from concourse.alu_op_type import AluOpType
from concourse.bass_utils import run_bass_kernel_spmd

B, D, H = 256, 1024, 2048
NCORES = 8
DSL = D // NCORES  # 128 d-columns per core
SIGMA1 = 0.02

F32 = mybir.dt.float32
BF16 = mybir.dt.bfloat16
FP8 = mybir.dt.float8e4
BFNP = ml_dtypes.bfloat16
F8NP = ml_dtypes.float8_e4m3

ERFA = float(2.0 / np.sqrt(np.pi))      # tanh scale
ERFC = float(0.10091075 / ERFA)          # z^3 coefficient (fit, err 3.6e-4)
LNPI2 = float(0.5 * np.log(np.pi))       # folded into exp(-z^2) bias
LN2 = float(np.log(2.0))

# fb blob (fp8, [2, 2, 3072]) offsets in the last dim. All bias matmuls
# run as K=4 fp8 DoubleRow so mm1/mm2 never switch dtype on the PE.
# Contraction rows are (p, r) pairs: (0,0), (1,0), (0,1), (1,1).
FB_TV = 0        # mm1 bias rhs:  (0,0)=t, (1,0)=ones
FB_W1T = 256     # mm1 bias lhsT: (0,0)=W1[D,:], (1,0)=b1
FB_LNW = 2304    # LN bias lhsT:  (0,0)=b2ln, others ones
FB_LNR = 2432    # LN bias rhs:   (0,0)=ones, then -lnCE hi/mid/lo fp8 split
FB_MUW = 2688    # MU bias lhsT:  (0,0)=b2mu
FB_MUR = 2816    # MU bias rhs:   (0,0)=ones
FB_W = 3072

# ep blob (f32, 128 partitions) column offsets
EP_S, EP_RM, EP_XC, EP_SQW, EP_ONE = 0, 256, 512, 768, 1024
EP_BLN2, EP_BPI = 1025, 1026   # bias columns: -ln2, -0.5*ln(pi)
EP_W = 1027


def _build(debug=False):
    nc = bacc.Bacc("TRN2", target_bir_lowering=False, debug=False,
                   num_devices=NCORES)

    d_muT = nc.dram_tensor("muT8", (128, 8 * B), FP8, kind="ExternalInput")
    d_w1 = nc.dram_tensor("w1m", (128, 16 * 8 * 128), FP8,
                          kind="ExternalInput")
    d_w2 = nc.dram_tensor("w2m", (128, 2 * 8 * 2 * 128), FP8,
                          kind="ExternalInput")
    d_fb = nc.dram_tensor("fb", (2, 2 * FB_W), FP8, kind="ExternalInput")
    d_ep = nc.dram_tensor("ep", (128, EP_W), F32, kind="ExternalInput")
    d_out = nc.dram_tensor("outp", (2, 1), F32, kind="ExternalOutput")

    MULT, ADD, SUB, BYP = (AluOpType.mult, AluOpType.add,
                           AluOpType.subtract, AluOpType.bypass)
    AF = mybir.ActivationFunctionType
    DR = mybir.MatmulPerfMode.DoubleRow

    with tile.TileContext(nc) as tc:
        with (
            tc.tile_pool(name="weights", bufs=1) as wpool,
            tc.tile_pool(name="work", bufs=1) as work,
        ):
            muT = wpool.tile([128, 8, B], FP8)
            w1s = [wpool.tile([128, 2, 8, 128], FP8, name=f"w1s{i}")
                   for i in range(8)]
            w2 = wpool.tile([128, 2, 8, 2, 128], FP8)
            fb = wpool.tile([2, 2, FB_W], FP8)
            eps = wpool.tile([128, EP_W], F32)
            jw = wpool.tile([128, 640], BF16)
            hT = work.tile([128, 16, B], FP8)

            with (
                tc.tile_pool(name="psJ", bufs=1,
                             space=bass.MemorySpace.PSUM) as psJ,
                tc.tile_pool(name="psA", bufs=4,
                             space=bass.MemorySpace.PSUM) as psA,
                tc.tile_pool(name="psO", bufs=1,
                             space=bass.MemorySpace.PSUM) as psO,
            ):
                # ---- input DMAs: sync (HWDGE) carries the mm1-critical
                # tensors in need-order. The mm2/epilogue tensors go on
                # gpsimd (SWDGE) but are deferred behind mm1's first tile
                # (dummy dep below) so they don't steal DMA-engine
                # bandwidth from the mm1-critical transfers.
                nc.gpsimd.memset(jw[:], 0.0)
                nc.sync.dma_start(w1s[0][:], d_w1.ap()[:, 0:2048])
                nc.sync.dma_start(muT[:], d_muT.ap()[:])
                nc.sync.dma_start(fb[:], d_fb.ap()[:])
                for s4 in range(1, 8):
                    nc.sync.dma_start(
                        w1s[s4][:], d_w1.ap()[:, s4 * 2048:(s4 + 1) * 2048])
                # mm2/epilogue tensors at the sync-queue tail: their
                # transfers start only after the mm1-critical ones, so
                # they don't steal DMA-engine bandwidth from them.
                nc.sync.dma_start(w2[:], d_w2.ap()[:])
                nc.sync.dma_start(eps[:], d_ep.ap()[:])

                # ---- PE warm-up: dense junk matmuls (no DMA deps) so HAM
                # lifts the 1.2GHz cold clock before mm1's data lands.
                jp = psJ.tile([128, 512], F32)
                for _ in range(12):
                    nc.tensor.matmul(jp[:], jw[:, 0:128], jw[:, 128:640],
                                     start=True, stop=True)

                # ---- mm1: hT[m] = PRelu(W1^T mu_cat^T) fp8 DoubleRow;
                # t-row and b1 folded in as a K=2 bf16 matmul.
                for m in range(16):
                    ph = psA.tile([128, B], F32, tag="ph")
                    for j in range(4):
                        nc.tensor.matmul(
                            ph[:], w1s[m // 2][:, m % 2, 2 * j:2 * j + 2, :],
                            muT[:, 2 * j:2 * j + 2, :],
                            start=(j == 0), stop=False, perf_mode=DR)
                    ms = slice(FB_W1T + m * 128, FB_W1T + (m + 1) * 128)
                    nc.tensor.matmul(ph[:], fb[:, :, ms],
                                     fb[:, :, FB_TV:FB_TV + B],
                                     start=False, stop=True, perf_mode=DR)
                    nc.scalar.activation(hT[:, m, :], ph[:], AF.Prelu,
                                         bias=0.0, scale=1.0, alpha=0.01)

                # ---- mm2: PMU = W2mu^T hT + b2mu first (so the a4/md
                # vector ops overlap the LN matmuls), then PLN = W2ln^T hT
                # + b2ln - lnCE (hi/lo bf16 rows). M=128, fp8 DoubleRow.
                pmu = psO.tile([128, B], F32, name="pmu")
                for j in range(8):
                    nc.tensor.matmul(pmu[:], w2[:, 0, j, :, :],
                                     hT[:, 2 * j:2 * j + 2, :],
                                     start=(j == 0), stop=False, perf_mode=DR)
                nc.tensor.matmul(pmu[:], fb[:, :, FB_MUW:FB_MUW + 128],
                                 fb[:, :, FB_MUR:FB_MUR + B],
                                 start=False, stop=True, perf_mode=DR)
                pln = psO.tile([128, B], F32, name="pln")
                for j in range(8):
                    nc.tensor.matmul(pln[:], w2[:, 1, j, :, :],
                                     hT[:, 2 * j:2 * j + 2, :],
                                     start=(j == 0), stop=False, perf_mode=DR)
                nc.tensor.matmul(pln[:], fb[:, :, FB_LNW:FB_LNW + 128],
                                 fb[:, :, FB_LNR:FB_LNR + B],
                                 start=False, stop=True, perf_mode=DR)

                # ---- epilogue: two column halves pipelined across
                # ACT/DVE/GpSimd. Sign trick: z*erf(z) and e^{-z^2} are
                # even, so we compute -z (saving the mu_x*inv op) and only
                # fix the sign of the standalone erf(z_127) term.
                HB = B // 2
                inv = work.tile([128, B], F32, name="inv")
                nc.scalar.activation(inv[:], pln[:], AF.Exp,
                                     bias=0.0, scale=-1.0)
                isp = work.tile([128, B], F32, name="isp")
                nc.scalar.activation(isp[:], pln[:], AF.Exp,
                                     bias=eps[:, EP_BLN2:EP_BLN2 + 1],
                                     scale=1.0)

                a4 = work.tile([128, B], F32, name="a4")
                md = work.tile([128, B], F32, name="md")
                zab = work.tile([128, 2, 3, HB], F32, name="zab")
                sq = work.tile([128, 2, 3, HB], F32, name="sq")
                cu = work.tile([128, 2, 3, HB], F32, name="cu")
                uu = work.tile([128, 2, 3, HB], F32, name="uu")
                erf = work.tile([128, 2, 3, HB], F32, name="erf")
                exg = work.tile([128, 2, 2, HB], F32, name="exg")
                a1 = work.tile([128, B], F32, name="a1")
                b1t = work.tile([128, B], F32, name="b1t")
                c1 = work.tile([128, B], F32, name="c1")
                d1 = work.tile([128, B], F32, name="d1")
                e1 = work.tile([128, B], F32, name="e1")
                sg = work.tile([128, B], F32, name="sg")
                gg = work.tile([128, B], F32, name="gg")
                hh = work.tile([128, B], F32, name="hh")
                hs = work.tile([128, B], F32, name="hs")
                h2 = work.tile([128, B], F32, name="h2")
                part = work.tile([128, 2], F32, name="part")

                def front_md(h):
                    bs = slice(h * HB, (h + 1) * HB)
                    nc.vector.tensor_tensor(
                        a4[:, bs], eps[:, EP_RM + h * HB:EP_RM + h * HB + HB],
                        pmu[:, bs], MULT)
                    nc.vector.tensor_tensor(
                        md[:, bs], eps[:, EP_S + h * HB:EP_S + h * HB + HB],
                        a4[:, bs], SUB)

                def front(h):
                    bs = slice(h * HB, (h + 1) * HB)
                    # zab holds -z: (-z_a, -z_b, -z_7)
                    for slot, c, op in ((0, 0.9921875, ADD),
                                        (1, 0.9921875, SUB),
                                        (2, 0.984375, SUB)):
                        nc.vector.scalar_tensor_tensor(
                            zab[:, h, slot, :], md[:, bs], c, inv[:, bs],
                            op0=op, op1=MULT)

                def cubic(h):
                    nc.vector.tensor_tensor(cu[:, h], sq[:, h], zab[:, h],
                                            MULT)
                    nc.vector.scalar_tensor_tensor(uu[:, h], cu[:, h], ERFC,
                                                   zab[:, h], op0=MULT,
                                                   op1=ADD)

                def tail(h):
                    bs = slice(h * HB, (h + 1) * HB)
                    nc.vector.tensor_tensor(a1[:, bs], zab[:, h, 1, :],
                                            erf[:, h, 1, :], MULT)
                    nc.vector.tensor_tensor(d1[:, bs], a1[:, bs], b1t[:, bs],
                                            SUB)
                    nc.vector.tensor_tensor(e1[:, bs], d1[:, bs], c1[:, bs],
                                            ADD)
                    nc.vector.tensor_tensor(sg[:, bs], e1[:, bs], isp[:, bs],
                                            MULT)
                    nc.vector.tensor_tensor(
                        gg[:, bs], sg[:, bs],
                        eps[:, EP_XC + h * HB:EP_XC + h * HB + HB], ADD)
                    nc.vector.scalar_tensor_tensor(
                        hh[:, bs], erf[:, h, 2, :], -127.0 / 256.0, gg[:, bs],
                        op0=MULT, op1=SUB)
                    nc.vector.tensor_tensor(
                        hs[:, bs], hh[:, bs],
                        eps[:, EP_SQW + h * HB:EP_SQW + h * HB + HB], MULT)
                    nc.vector.scalar_tensor_tensor(
                        h2[:, bs], hs[:, bs], 1.0, hs[:, bs],
                        op0=BYP, op1=MULT, accum_out=part[:, h:h + 1])

                def gp_side(h):
                    bs = slice(h * HB, (h + 1) * HB)
                    nc.gpsimd.tensor_tensor(b1t[:, bs], zab[:, h, 0, :],
                                            erf[:, h, 0, :], MULT)
                    nc.gpsimd.tensor_tensor(c1[:, bs], exg[:, h, 1, :],
                                            exg[:, h, 0, :], SUB)

                front_md(0)
                front_md(1)
                front(0)
                front(1)
                nc.scalar.activation(sq[:, 0], zab[:, 0], AF.Square,
                                     bias=0.0, scale=1.0)
                cubic(0)
                nc.scalar.activation(erf[:, 0], uu[:, 0], AF.Tanh,
                                     bias=0.0, scale=ERFA)
                nc.scalar.activation(sq[:, 1], zab[:, 1], AF.Square,
                                     bias=0.0, scale=1.0)
                nc.scalar.activation(exg[:, 0], sq[:, 0, 0:2, :], AF.Exp,
                                     bias=eps[:, EP_BPI:EP_BPI + 1],
                                     scale=-1.0)
                cubic(1)
                nc.scalar.activation(erf[:, 1], uu[:, 1], AF.Tanh,
                                     bias=0.0, scale=ERFA)
                nc.scalar.activation(exg[:, 1], sq[:, 1, 0:2, :], AF.Exp,
                                     bias=eps[:, EP_BPI:EP_BPI + 1],
                                     scale=-1.0)
                gp_side(0)
                tail(0)
                gp_side(1)
                tail(1)

                # cross-partition reduce: [2,1] psum via ones-matmul, so
                # the output DMA is two 4-byte descriptors.
                ps1 = psO.tile([2, 1], F32, name="ps1")
                nc.tensor.matmul(ps1[:], part[:],
                                 eps[:, EP_ONE:EP_ONE + 1],
                                 start=True, stop=True)
                sres = work.tile([2, 1], F32, name="sres")
                nc.vector.tensor_copy(sres[:], ps1[:])
                nc.sync.dma_start(d_out.ap()[:], sres[:])

    nc.compile()
    return nc


def host_prep(x, t, noise, W1, b1, W2, b2):
    """Build the per-core in_maps (host-side packing + tiny per-row math)."""
    f32 = np.float32
    tv = t[:, 0].astype(f32)
    assert float(tv.min()) > 1e-8, "low-t mask path not implemented"
    gamma = (1.0 - np.power(f32(SIGMA1), f32(2.0) * tv)).astype(f32)
    assert float(gamma.min()) > 0.0
    r = np.sqrt((1.0 - gamma) / gamma).astype(f32)
    lnce = np.log(1.0 / (r * np.sqrt(f32(2.0)))).astype(f32)
    v = (-lnce).astype(f32)
    nlh = v.astype(F8NP)
    nlm = (v - nlh.astype(f32)).astype(F8NP)
    nll = (v - nlh.astype(f32) - nlm.astype(f32)).astype(F8NP)
    sqw = np.power(f32(SIGMA1), -tv).astype(f32)

    mu = (gamma[:, None] * x + (gamma * (1 - gamma))[:, None] * noise
          ).astype(f32)
    muT8 = np.ascontiguousarray(
        mu.T.reshape(8, 128, B).transpose(1, 0, 2).reshape(128, 8 * B)
        .astype(F8NP))

    # w1m[p, (m*8+k)*128 + c] = W1[k*128+p, m*128+c]
    w1f = W1[:D].astype(f32).reshape(8, 128, 16, 128)
    w1m = np.ascontiguousarray(
        w1f.transpose(1, 2, 0, 3).reshape(128, 16 * 8 * 128).astype(F8NP))

    fbv = np.zeros((2, 2, FB_W), dtype=F8NP)
    fbv[0, 0, FB_TV:FB_TV + B] = tv.astype(F8NP)
    fbv[1, 0, FB_TV:FB_TV + B] = F8NP(1.0)
    fbv[0, 0, FB_W1T:FB_W1T + H] = W1[D].astype(F8NP)
    fbv[1, 0, FB_W1T:FB_W1T + H] = b1.astype(F8NP)
    fbv[1, 0, FB_LNW:FB_LNW + 128] = F8NP(1.0)
    fbv[0, 1, FB_LNW:FB_LNW + 128] = F8NP(1.0)
    fbv[1, 1, FB_LNW:FB_LNW + 128] = F8NP(1.0)
    fbv[0, 0, FB_LNR:FB_LNR + B] = F8NP(1.0)
    fbv[1, 0, FB_LNR:FB_LNR + B] = nlh
    fbv[0, 1, FB_LNR:FB_LNR + B] = nlm
    fbv[1, 1, FB_LNR:FB_LNR + B] = nll
    fbv[0, 0, FB_MUR:FB_MUR + B] = F8NP(1.0)

    epv = np.zeros((128, EP_W), dtype=f32)
    epv[:, EP_RM:EP_RM + B] = r[None, :]
    epv[:, EP_SQW:EP_SQW + B] = sqw[None, :]
    epv[:, EP_ONE] = 1.0
    epv[:, EP_BLN2] = -LN2
    epv[:, EP_BPI] = -LNPI2

    S_full = (x + (1.0 - gamma)[:, None] * noise).astype(f32)
    XC_full = (x + f32(127.0 / 256.0)).astype(f32)

    in_maps = []
    for i in range(NCORES):
        dsl = slice(i * DSL, (i + 1) * DSL)
        # w2m[p, ((half*8+j)*2+rr)*128+c] = W2[(2j+rr)*128+p, half*D + col]
        w2c = np.stack([W2[:, dsl], W2[:, D + i * DSL:D + (i + 1) * DSL]],
                       axis=0)  # [2, 2048, 128]
        w2m = np.ascontiguousarray(
            w2c.reshape(2, 16, 128, 128).transpose(2, 0, 1, 3)
            .reshape(128, 2 * 16 * 128).astype(F8NP))
        fbi = fbv.copy()
        fbi[0, 0, FB_LNW:FB_LNW + 128] = b2[D + i * DSL:D + (i + 1) * DSL
                                            ].astype(F8NP)
        fbi[0, 0, FB_MUW:FB_MUW + 128] = b2[dsl].astype(F8NP)
        epi = epv.copy()
        epi[:, EP_S:EP_S + B] = S_full[:, dsl].T
        epi[:, EP_XC:EP_XC + B] = XC_full[:, dsl].T
        in_maps.append({
            "muT8": muT8, "w1m": w1m, "w2m": w2m,
            "fb": fbi.reshape(2, 2 * FB_W), "ep": epi,
        })
    return in_maps


_nc_cache = {}


def get_nc(debug=False):
    if debug not in _nc_cache:
        _nc_cache[debug] = _build(debug)
    return _nc_cache[debug]


def run_on_cores(inputs, trace=False, debug=False, tmpdir=None):
    nc = get_nc(debug)
    in_maps = host_prep(**inputs)
    res = run_bass_kernel_spmd(nc, in_maps, core_ids=list(range(NCORES)),
                               trace=trace, tmpdir=tmpdir)
    total = np.float64(0.0)
    for i in range(NCORES):
        total += res.results[i]["outp"].astype(np.float64).sum()
    loss = np.float32(-np.log(np.float32(SIGMA1)) * total / float(B * D))
    return loss, res


_reset_done = [False]


def _maybe_reset_device():
    # Clear a wedged NRT exec unit left by a previous process. Best-effort.
    if _reset_done[0]:
        return
    _reset_done[0] = True
    try:
        import os
        import ctypes
        so = "/opt/axon/libaxon_pjrt.so"
        if os.path.exists(so):
            import jax

            jax.devices()
            lib = ctypes.CDLL(so)
            lib.axon_reset.restype = ctypes.c_int64
            lib.axon_reset()
    except Exception:
        pass


def kernel(**inputs):
    _maybe_reset_device()
    inputs = {k: np.asarray(v) for k, v in inputs.items()}
    loss, _ = run_on_cores(inputs)
    return np.asarray(loss, dtype=np.float32)


# revision 33
# speedup vs baseline: 2.4743x; 1.0461x over previous
"""Trainium2 Bass kernel for nn_DiscretisedBNF (discretised BNF loss).

Math: the reference's (B, D, K=128) clamped-CDF bin sum Abel-collapses to

    pO[b,d] = -127/256 - (1/128)*Sigma + (127/256)*erf(z_127),
    Sigma   = sum_{k=1..127} erf(z_k),  z_k = (e_k - mu_x)*inv

and Sigma is a uniform-grid Riemann sum of erf, so by Poisson summation
it equals the midpoint integral up to O(exp(-pi^2/s^2)) (s = inv/64):

    Sigma ~= (1/s)[ z_b*erf(z_b) - z_a*erf(z_a) + (e^{-z_b^2}-e^{-z_a^2})/sqrt(pi) ]
    z_a = inv*(-0.9921875) - mu_x*inv,  z_b = inv*(0.9921875) - mu_x*inv

This replaces the whole 127-bin binning phase (4.2M tanh + z/q matmuls
per core) with ~5 ACT passes and ~15 vector ops per [128,256] tile.
erf is evaluated as tanh((2/sqrt(pi))*(z + c*z^3)) (max abs err 3.6e-4),
so exp+tanh+square+prelu all live in the one resident ACT table set
(exp_and_others) -- no table switches.  End-to-end numpy mirror of the
device math (incl. fp8/bf16 quantization): rel err ~8e-5.

Constant foldings: mm2's ln-tile bias rows add -ln(cexp) (hi/lo bf16
split) so that  inv = exp(-PLN)  and  1/(128 s) = exp(PLN - ln2)  come
straight out of ACT with scalar biases; 1/sqrt(pi) is folded into the
exp bias.

Sharding (8 cores, full inputs in, full output out): mm1 replicated
(fp8 DoubleRow), W2 column-sharded 128+128 cols per core, epilogue
data-parallel on the core's [128 d x 256 b] tile. Output is a single
f32 partial per core (cross-partition reduce via a ones-matmul) so the
final DMA is one 4-byte descriptor. Host sums 8 partials.

PE warm-up: ~8 junk N=512 matmuls on a memset tile right at kernel
start keep HAM from running mm1 at the cold 1.2 GHz clock.
"""

import sys

sys.path.insert(0, "/opt/trn_rl_repo")

import numpy as np
import ml_dtypes

import concourse.bass as bass
import concourse.tile as tile
from concourse import bacc, mybir
from concourse.alu_op_type import AluOpType
from concourse.bass_utils import run_bass_kernel_spmd

B, D, H = 256, 1024, 2048
NCORES = 8
DSL = D // NCORES  # 128 d-columns per core
SIGMA1 = 0.02

F32 = mybir.dt.float32
BF16 = mybir.dt.bfloat16
FP8 = mybir.dt.float8e4
BFNP = ml_dtypes.bfloat16
F8NP = ml_dtypes.float8_e4m3

ERFA = float(2.0 / np.sqrt(np.pi))      # tanh scale
ERFC = float(0.10091075 / ERFA)          # z^3 coefficient (fit, err 3.6e-4)
LNPI2 = float(0.5 * np.log(np.pi))       # folded into exp(-z^2) bias
LN2 = float(np.log(2.0))

# fb blob (fp8, [2, 2, 3072]) offsets in the last dim. All bias matmuls
# run as K=4 fp8 DoubleRow so mm1/mm2 never switch dtype on the PE.
# Contraction rows are (p, r) pairs: (0,0), (1,0), (0,1), (1,1).
FB_TV = 0        # mm1 bias rhs:  (0,0)=t, (1,0)=ones
FB_W1T = 256     # mm1 bias lhsT: (0,0)=W1[D,:], (1,0)=b1
FB_LNW = 2304    # LN bias lhsT:  (0,0)=b2ln, others ones
FB_LNR = 2432    # LN bias rhs:   (0,0)=ones, then -lnCE hi/mid/lo fp8 split
FB_MUW = 2688    # MU bias lhsT:  (0,0)=b2mu
FB_MUR = 2816    # MU bias rhs:   (0,0)=ones
FB_W = 3072

# ep blob (f32, 128 partitions) column offsets
EP_S, EP_RM, EP_XC, EP_SQW, EP_ONE = 0, 256, 512, 768, 1024
EP_BLN2, EP_BPI = 1025, 1026   # bias columns: -ln2, -0.5*ln(pi)
EP_W = 1027


def _build(debug=False):
    nc = bacc.Bacc("TRN2", target_bir_lowering=False, debug=False,
                   num_devices=NCORES)

    d_muT = nc.dram_tensor("muT8", (128, 8 * B), FP8, kind="ExternalInput")
    d_w1 = nc.dram_tensor("w1m", (128, 16 * 8 * 128), FP8,
                          kind="ExternalInput")
    d_w2 = nc.dram_tensor("w2m", (128, 2 * 8 * 2 * 128), FP8,
                          kind="ExternalInput")
    d_fb = nc.dram_tensor("fb", (2, 2 * FB_W), FP8, kind="ExternalInput")
    d_ep = nc.dram_tensor("ep", (128, EP_W), F32, kind="ExternalInput")
    d_out = nc.dram_tensor("outp", (2, 1), F32, kind="ExternalOutput")

    MULT, ADD, SUB, BYP = (AluOpType.mult, AluOpType.add,
                           AluOpType.subtract, AluOpType.bypass)
    AF = mybir.ActivationFunctionType
    DR = mybir.MatmulPerfMode.DoubleRow

    with tile.TileContext(nc) as tc:
        with (
            tc.tile_pool(name="weights", bufs=1) as wpool,
            tc.tile_pool(name="work", bufs=1) as work,
        ):
            muT = wpool.tile([128, 8, B], FP8)
            w1s = [wpool.tile([128, 2, 8, 128], FP8, name=f"w1s{i}")
                   for i in range(8)]
            w2 = wpool.tile([128, 2, 8, 2, 128], FP8)
            fb = wpool.tile([2, 2, FB_W], FP8)
            eps = wpool.tile([128, EP_W], F32)
            jw = wpool.tile([128, 640], BF16)
            hT = work.tile([128, 16, B], FP8)

            with (
                tc.tile_pool(name="psJ", bufs=1,
                             space=bass.MemorySpace.PSUM) as psJ,
                tc.tile_pool(name="psA", bufs=4,
                             space=bass.MemorySpace.PSUM) as psA,
                tc.tile_pool(name="psO", bufs=1,
                             space=bass.MemorySpace.PSUM) as psO,
            ):
                # ---- input DMAs: sync (HWDGE) carries the mm1-critical
                # tensors in need-order. The mm2/epilogue tensors go on
                # gpsimd (SWDGE) but are deferred behind mm1's first tile
                # (dummy dep below) so they don't steal DMA-engine
                # bandwidth from the mm1-critical transfers.
                nc.gpsimd.memset(jw[:], 0.0)
                nc.sync.dma_start(w1s[0][:], d_w1.ap()[:, 0:2048])
                nc.sync.dma_start(muT[:], d_muT.ap()[:])
                nc.sync.dma_start(fb[:], d_fb.ap()[:])
                for s4 in range(1, 8):
                    nc.sync.dma_start(
                        w1s[s4][:], d_w1.ap()[:, s4 * 2048:(s4 + 1) * 2048])
                # mm2/epilogue tensors at the sync-queue tail: their
                # transfers start only after the mm1-critical ones, so
                # they don't steal DMA-engine bandwidth from them.
                nc.sync.dma_start(w2[:], d_w2.ap()[:])
                nc.sync.dma_start(eps[:], d_ep.ap()[:])

                # ---- PE warm-up: dense junk matmuls (no DMA deps) so HAM
                # lifts the 1.2GHz cold clock before mm1's data lands.
                jp = psJ.tile([128, 512], F32)
                for _ in range(8):
                    nc.tensor.matmul(jp[:], jw[:, 0:128], jw[:, 128:640],
                                     start=True, stop=True)

                # ---- mm1: hT[m] = PRelu(W1^T mu_cat^T) fp8 DoubleRow;
                # t-row and b1 folded in as a K=2 bf16 matmul.
                for m in range(16):
                    ph = psA.tile([128, B], F32, tag="ph")
                    for j in range(4):
                        nc.tensor.matmul(
                            ph[:], w1s[m // 2][:, m % 2, 2 * j:2 * j + 2, :],
                            muT[:, 2 * j:2 * j + 2, :],
                            start=(j == 0), stop=False, perf_mode=DR)
                    ms = slice(FB_W1T + m * 128, FB_W1T + (m + 1) * 128)
                    nc.tensor.matmul(ph[:], fb[:, :, ms],
                                     fb[:, :, FB_TV:FB_TV + B],
                                     start=False, stop=True, perf_mode=DR)
                    nc.scalar.activation(hT[:, m, :], ph[:], AF.Prelu,
                                         bias=0.0, scale=1.0, alpha=0.01)

                # ---- mm2: PMU = W2mu^T hT + b2mu first (so the a4/md
                # vector ops overlap the LN matmuls), then PLN = W2ln^T hT
                # + b2ln - lnCE (hi/lo bf16 rows). M=128, fp8 DoubleRow.
                pmu = psO.tile([128, B], F32, name="pmu")
                for j in range(8):
                    nc.tensor.matmul(pmu[:], w2[:, 0, j, :, :],
                                     hT[:, 2 * j:2 * j + 2, :],
                                     start=(j == 0), stop=False, perf_mode=DR)
                nc.tensor.matmul(pmu[:], fb[:, :, FB_MUW:FB_MUW + 128],
                                 fb[:, :, FB_MUR:FB_MUR + B],
                                 start=False, stop=True, perf_mode=DR)
                pln = psO.tile([128, B], F32, name="pln")
                for j in range(8):
                    nc.tensor.matmul(pln[:], w2[:, 1, j, :, :],
                                     hT[:, 2 * j:2 * j + 2, :],
                                     start=(j == 0), stop=False, perf_mode=DR)
                nc.tensor.matmul(pln[:], fb[:, :, FB_LNW:FB_LNW + 128],
                                 fb[:, :, FB_LNR:FB_LNR + B],
                                 start=False, stop=True, perf_mode=DR)

                # ---- epilogue: two column halves pipelined across
                # ACT/DVE/GpSimd. Sign trick: z*erf(z) and e^{-z^2} are
                # even, so we compute -z (saving the mu_x*inv op) and only
                # fix the sign of the standalone erf(z_127) term.
                HB = B // 2
                inv = work.tile([128, B], F32, name="inv")
                nc.scalar.activation(inv[:], pln[:], AF.Exp,
                                     bias=0.0, scale=-1.0)
                isp = work.tile([128, B], F32, name="isp")

                a4 = work.tile([128, B], F32, name="a4")
                md = work.tile([128, B], F32, name="md")
                zab = work.tile([128, 2, 3, HB], F32, name="zab")
                sq = work.tile([128, 2, 3, HB], F32, name="sq")
                cu = work.tile([128, 2, 3, HB], F32, name="cu")
                uu = work.tile([128, 2, 3, HB], F32, name="uu")
                erf = work.tile([128, 2, 3, HB], F32, name="erf")
                exg = work.tile([128, 2, 2, HB], F32, name="exg")
                a1 = work.tile([128, B], F32, name="a1")
                b1t = work.tile([128, B], F32, name="b1t")
                c1 = work.tile([128, B], F32, name="c1")
                d1 = work.tile([128, B], F32, name="d1")
                e1 = work.tile([128, B], F32, name="e1")
                sg = work.tile([128, B], F32, name="sg")
                gg = work.tile([128, B], F32, name="gg")
                hh = work.tile([128, B], F32, name="hh")
                hs = work.tile([128, B], F32, name="hs")
                h2 = work.tile([128, B], F32, name="h2")
                part = work.tile([128, 2], F32, name="part")

                def front_md(h):
                    bs = slice(h * HB, (h + 1) * HB)
                    nc.vector.tensor_tensor(
                        a4[:, bs], eps[:, EP_RM + h * HB:EP_RM + h * HB + HB],
                        pmu[:, bs], MULT)
                    nc.vector.tensor_tensor(
                        md[:, bs], eps[:, EP_S + h * HB:EP_S + h * HB + HB],
                        a4[:, bs], SUB)

                def front(h):
                    bs = slice(h * HB, (h + 1) * HB)
                    # zab holds -z: (-z_a, -z_b, -z_7)
                    for slot, c, op in ((0, 0.9921875, ADD),
                                        (1, 0.9921875, SUB),
                                        (2, 0.984375, SUB)):
                        nc.vector.scalar_tensor_tensor(
                            zab[:, h, slot, :], md[:, bs], c, inv[:, bs],
                            op0=op, op1=MULT)

                def cubic(h):
                    nc.vector.tensor_tensor(cu[:, h], sq[:, h], zab[:, h],
                                            MULT)
                    nc.vector.scalar_tensor_tensor(uu[:, h], cu[:, h], ERFC,
                                                   zab[:, h], op0=MULT,
                                                   op1=ADD)

                def tail(h):
                    bs = slice(h * HB, (h + 1) * HB)
                    nc.vector.tensor_tensor(a1[:, bs], zab[:, h, 1, :],
                                            erf[:, h, 1, :], MULT)
                    nc.vector.tensor_tensor(d1[:, bs], a1[:, bs], b1t[:, bs],
                                            SUB)
                    nc.vector.tensor_tensor(e1[:, bs], d1[:, bs], c1[:, bs],
                                            ADD)
                    nc.vector.tensor_tensor(sg[:, bs], e1[:, bs], isp[:, bs],
                                            MULT)
                    nc.vector.tensor_tensor(
                        gg[:, bs], sg[:, bs],
                        eps[:, EP_XC + h * HB:EP_XC + h * HB + HB], ADD)
                    nc.vector.scalar_tensor_tensor(
                        hh[:, bs], erf[:, h, 2, :], -127.0 / 256.0, gg[:, bs],
                        op0=MULT, op1=SUB)
                    nc.vector.tensor_tensor(
                        hs[:, bs], hh[:, bs],
                        eps[:, EP_SQW + h * HB:EP_SQW + h * HB + HB], MULT)
                    nc.vector.scalar_tensor_tensor(
                        h2[:, bs], hs[:, bs], 1.0, hs[:, bs],
                        op0=BYP, op1=MULT, accum_out=part[:, h:h + 1])

                def gp_side(h):
                    bs = slice(h * HB, (h + 1) * HB)
                    nc.gpsimd.tensor_tensor(b1t[:, bs], zab[:, h, 0, :],
                                            erf[:, h, 0, :], MULT)
                    nc.gpsimd.tensor_tensor(c1[:, bs], exg[:, h, 1, :],
                                            exg[:, h, 0, :], SUB)

                front_md(0)
                front_md(1)
                front(0)
                front(1)
                nc.scalar.activation(sq[:, 0], zab[:, 0], AF.Square,
                                     bias=0.0, scale=1.0)
                cubic(0)
                nc.scalar.activation(erf[:, 0], uu[:, 0], AF.Tanh,
                                     bias=0.0, scale=ERFA)
                nc.scalar.activation(sq[:, 1], zab[:, 1], AF.Square,
                                     bias=0.0, scale=1.0)
                nc.scalar.activation(exg[:, 0], sq[:, 0, 0:2, :], AF.Exp,
                                     bias=eps[:, EP_BPI:EP_BPI + 1],
                                     scale=-1.0)
                cubic(1)
                nc.scalar.activation(erf[:, 1], uu[:, 1], AF.Tanh,
                                     bias=0.0, scale=ERFA)
                nc.scalar.activation(exg[:, 1], sq[:, 1, 0:2, :], AF.Exp,
                                     bias=eps[:, EP_BPI:EP_BPI + 1],
                                     scale=-1.0)
                # isp is only consumed by the late sg step; emit it after
                # the ladder so it doesn't delay SQ/TANH on the ACT queue.
                nc.scalar.activation(isp[:], pln[:], AF.Exp,
                                     bias=eps[:, EP_BLN2:EP_BLN2 + 1],
                                     scale=1.0)
                gp_side(0)
                tail(0)
                gp_side(1)
                tail(1)

                # cross-partition reduce: [2,1] psum via ones-matmul, so
                # the output DMA is two 4-byte descriptors.
                ps1 = psO.tile([2, 1], F32, name="ps1")
                nc.tensor.matmul(ps1[:], part[:],
                                 eps[:, EP_ONE:EP_ONE + 1],
                                 start=True, stop=True)
                sres = work.tile([2, 1], F32, name="sres")
                nc.vector.tensor_copy(sres[:], ps1[:])
                nc.sync.dma_start(d_out.ap()[:], sres[:])

    nc.compile()
    return nc


def host_prep(x, t, noise, W1, b1, W2, b2):
    """Build the per-core in_maps (host-side packing + tiny per-row math)."""
    f32 = np.float32
    tv = t[:, 0].astype(f32)
    assert float(tv.min()) > 1e-8, "low-t mask path not implemented"
    gamma = (1.0 - np.power(f32(SIGMA1), f32(2.0) * tv)).astype(f32)
    assert float(gamma.min()) > 0.0
    r = np.sqrt((1.0 - gamma) / gamma).astype(f32)
    lnce = np.log(1.0 / (r * np.sqrt(f32(2.0)))).astype(f32)
    v = (-lnce).astype(f32)
    nlh = v.astype(F8NP)
    nlm = (v - nlh.astype(f32)).astype(F8NP)
    nll = (v - nlh.astype(f32) - nlm.astype(f32)).astype(F8NP)
    sqw = np.power(f32(SIGMA1), -tv).astype(f32)

    mu = (gamma[:, None] * x + (gamma * (1 - gamma))[:, None] * noise
          ).astype(f32)
    muT8 = np.ascontiguousarray(
        mu.T.reshape(8, 128, B).transpose(1, 0, 2).reshape(128, 8 * B)
        .astype(F8NP))

    # w1m[p, (m*8+k)*128 + c] = W1[k*128+p, m*128+c]
    w1f = W1[:D].astype(f32).reshape(8, 128, 16, 128)
    w1m = np.ascontiguousarray(
        w1f.transpose(1, 2, 0, 3).reshape(128, 16 * 8 * 128).astype(F8NP))

    fbv = np.zeros((2, 2, FB_W), dtype=F8NP)
    fbv[0, 0, FB_TV:FB_TV + B] = tv.astype(F8NP)
    fbv[1, 0, FB_TV:FB_TV + B] = F8NP(1.0)
    fbv[0, 0, FB_W1T:FB_W1T + H] = W1[D].astype(F8NP)
    fbv[1, 0, FB_W1T:FB_W1T + H] = b1.astype(F8NP)
    fbv[1, 0, FB_LNW:FB_LNW + 128] = F8NP(1.0)
    fbv[0, 1, FB_LNW:FB_LNW + 128] = F8NP(1.0)
    fbv[1, 1, FB_LNW:FB_LNW + 128] = F8NP(1.0)
    fbv[0, 0, FB_LNR:FB_LNR + B] = F8NP(1.0)
    fbv[1, 0, FB_LNR:FB_LNR + B] = nlh
    fbv[0, 1, FB_LNR:FB_LNR + B] = nlm
    fbv[1, 1, FB_LNR:FB_LNR + B] = nll
    fbv[0, 0, FB_MUR:FB_MUR + B] = F8NP(1.0)

    epv = np.zeros((128, EP_W), dtype=f32)
    epv[:, EP_RM:EP_RM + B] = r[None, :]
    epv[:, EP_SQW:EP_SQW + B] = sqw[None, :]
    epv[:, EP_ONE] = 1.0
    epv[:, EP_BLN2] = -LN2
    epv[:, EP_BPI] = -LNPI2

    S_full = (x + (1.0 - gamma)[:, None] * noise).astype(f32)
    XC_full = (x + f32(127.0 / 256.0)).astype(f32)

    in_maps = []
    for i in range(NCORES):
        dsl = slice(i * DSL, (i + 1) * DSL)
        # w2m[p, ((half*8+j)*2+rr)*128+c] = W2[(2j+rr)*128+p, half*D + col]
        w2c = np.stack([W2[:, dsl], W2[:, D + i * DSL:D + (i + 1) * DSL]],
                       axis=0)  # [2, 2048, 128]
        w2m = np.ascontiguousarray(
            w2c.reshape(2, 16, 128, 128).transpose(2, 0, 1, 3)
            .reshape(128, 2 * 16 * 128).astype(F8NP))
        fbi = fbv.copy()
        fbi[0, 0, FB_LNW:FB_LNW + 128] = b2[D + i * DSL:D + (i + 1) * DSL
                                            ].astype(F8NP)
        fbi[0, 0, FB_MUW:FB_MUW + 128] = b2[dsl].astype(F8NP)
        epi = epv.copy()
        epi[:, EP_S:EP_S + B] = S_full[:, dsl].T
        epi[:, EP_XC:EP_XC + B] = XC_full[:, dsl].T
        in_maps.append({
            "muT8": muT8, "w1m": w1m, "w2m": w2m,
            "fb": fbi.reshape(2, 2 * FB_W), "ep": epi,
        })
    return in_maps


_nc_cache = {}


def get_nc(debug=False):
    if debug not in _nc_cache:
        _nc_cache[debug] = _build(debug)
    return _nc_cache[debug]


def run_on_cores(inputs, trace=False, debug=False, tmpdir=None):
    nc = get_nc(debug)
    in_maps = host_prep(**inputs)
    res = run_bass_kernel_spmd(nc, in_maps, core_ids=list(range(NCORES)),
                               trace=trace, tmpdir=tmpdir)
    total = np.float64(0.0)
    for i in range(NCORES):
        total += res.results[i]["outp"].astype(np.float64).sum()
    loss = np.float32(-np.log(np.float32(SIGMA1)) * total / float(B * D))
    return loss, res


_reset_done = [False]


def _maybe_reset_device():
    # Clear a wedged NRT exec unit left by a previous process. Best-effort.
    if _reset_done[0]:
        return
    _reset_done[0] = True
    try:
        import os
        import ctypes
        so = "/opt/axon/libaxon_pjrt.so"
        if os.path.exists(so):
            import jax

            jax.devices()
            lib = ctypes.CDLL(so)
            lib.axon_reset.restype = ctypes.c_int64
            lib.axon_reset()
    except Exception:
        pass


def kernel(**inputs):
    _maybe_reset_device()
    inputs = {k: np.asarray(v) for k, v in inputs.items()}
    loss, _ = run_on_cores(inputs)
    return np.asarray(loss, dtype=np.float32)
